# revision 1
# baseline (speedup 1.0000x reference)
"""Trainium2 Bass kernel for nn_AttnBlock (GroupNorm + single-head 4096-token
attention + residual), sharded over 8 NeuronCores.

Sharding: data-parallel over batch B=4, sequence-parallel x2 over the 4096
query tokens -> 8 shards. Each core computes k/v for its full batch
(duplicated across the 2 token-halves) and q/attention/out-proj for its 2048
query tokens. The token axis is rolled on the host for the second half so a
single SPMD NEFF serves all cores (softmax over keys is order-invariant,
groupnorm stats are token-permutation-invariant).

Self-contained: hardcodes all shapes; only needs the concourse runtime.
"""

import numpy as np
import ml_dtypes

import concourse.bass as bass
import concourse.bacc as bacc
import concourse.tile as tile
from concourse import mybir
from concourse.bass_utils import run_bass_kernel_spmd

P = 128                 # partitions
C = 512                 # channels
N = 4096                # tokens (64*64)
NQ = 2048               # query tokens per core
CT = C // P             # 4 channel tiles
JT = N // P             # 32 key-token tiles of 128
NSTRIP = NQ // 512      # 4 query strips of 512
ITS = 512 // P          # 4 i-subtiles per strip
GS = 16                 # channels per group
NG = P // GS            # 8 groups per channel tile
EPS = 1e-6
SCALE = float(C) ** -0.5
F32 = mybir.dt.float32
BF16 = mybir.dt.bfloat16

_CACHE = {}


def build_bass():
    nc = bacc.Bacc(None, target_bir_lowering=False)

    x_h = nc.dram_tensor("x", [C, N], F32, kind="ExternalInput")[:]
    wq_h = nc.dram_tensor("wqT", [C, C], BF16, kind="ExternalInput")[:]
    wk_h = nc.dram_tensor("wkT", [C, C], BF16, kind="ExternalInput")[:]
    wv_h = nc.dram_tensor("wvT", [C, C], BF16, kind="ExternalInput")[:]
    wo_h = nc.dram_tensor("woT", [C, C], BF16, kind="ExternalInput")[:]
    bq_h = nc.dram_tensor("bq", [C], F32, kind="ExternalInput")[:]
    bk_h = nc.dram_tensor("bk", [C], F32, kind="ExternalInput")[:]
    bv_h = nc.dram_tensor("bv", [C], F32, kind="ExternalInput")[:]
    bo_h = nc.dram_tensor("bo", [C], F32, kind="ExternalInput")[:]
    gam_h = nc.dram_tensor("gam", [C], F32, kind="ExternalInput")[:]
    bet_h = nc.dram_tensor("bet", [C], F32, kind="ExternalInput")[:]
    out_h = nc.dram_tensor("out", [C, NQ], F32, kind="ExternalOutput")[:]

    g8_np = np.zeros((P, NG), np.float32)
    g8T_np = np.zeros((NG, P), np.float32)
    for c in range(P):
        g8_np[c, c // GS] = 1.0 / GS
        g8T_np[c // GS, c] = 1.0
    g8_h = nc.inline_tensor(g8_np, name="g8")[:]
    g8T_h = nc.inline_tensor(g8T_np, name="g8T")[:]

    x_t = x_h.rearrange("(t p) n -> t p n", p=P)          # [4,128,4096]
    out_t = out_h.rearrange("(t p) n -> t p n", p=P)      # [4,128,2048]

    def col4(ap1d):
        # [512] dram vector -> [128,4] sbuf layout (column ct holds chans ct*128..)
        return bass.AP(tensor=ap1d.tensor, offset=ap1d.offset, ap=[[1, P], [P, CT]])

    with tile.TileContext(nc) as tc:
        with tc.tile_pool(name="consts", bufs=1) as cp, \
             tc.tile_pool(name="wo", bufs=1) as wop, \
             tc.tile_pool(name="qkv", bufs=1) as qkvp, \
             tc.tile_pool(name="hT", bufs=1) as hTp, \
             tc.tile_pool(name="mm", bufs=3, space="PSUM") as mmp:

            # ---- constants ----
            ones_f32 = cp.tile([P, 1], F32, tag="ones")
            nc.vector.memset(ones_f32[:], 1.0)
            ones1 = cp.tile([1, P], F32, tag="ones1")
            nc.vector.memset(ones1[:], 1.0)
            eps_t = cp.tile([P, 1], F32, tag="eps")
            nc.vector.memset(eps_t[:], EPS)
            g8_sb = cp.tile([P, NG], F32, tag="g8")
            nc.sync.dma_start(out=g8_sb[:], in_=g8_h)
            g8T_sb = cp.tile([NG, P], F32, tag="g8T")
            nc.sync.dma_start(out=g8T_sb[:], in_=g8T_h)
            bq_sb = cp.tile([P, CT], F32, tag="bq")
            nc.sync.dma_start(out=bq_sb[:], in_=col4(bq_h))
            bk_sb = cp.tile([P, CT], F32, tag="bk")
            nc.sync.dma_start(out=bk_sb[:], in_=col4(bk_h))
            bo_sb = cp.tile([P, CT], F32, tag="bo")
            nc.sync.dma_start(out=bo_sb[:], in_=col4(bo_h))
            gam_sb = cp.tile([P, CT], F32, tag="gam")
            nc.sync.dma_start(out=gam_sb[:], in_=col4(gam_h))
            bet_sb = cp.tile([P, CT], F32, tag="bet")
            nc.sync.dma_start(out=bet_sb[:], in_=col4(bet_h))
            bv_bc = cp.tile([P, C], F32, tag="bvbc")

            # ---- persistent weights / activations ----
            wo_sb = [wop.tile([P, C], BF16, tag=f"wo{t}", name=f"wo{t}") for t in range(CT)]
            wo_t = wo_h.rearrange("(t p) o -> t p o", p=P)

            q_bf = [qkvp.tile([P, NQ], BF16, tag=f"q{t}", name=f"q{t}") for t in range(CT)]
            k_bf = [qkvp.tile([P, N], BF16, tag=f"k{t}", name=f"k{t}") for t in range(CT)]
            v_bf = [qkvp.tile([P, C], BF16, tag=f"v{j}", name=f"v{j}") for j in range(JT)]
            hT_bf = [hTp.tile([P, NQ], BF16, tag=f"hT{t}", name=f"hT{t}") for t in range(CT)]

            # =========== Phase A: groupnorm -> hn (bf16), and QKV ===========
            with tc.tile_pool(name="xg", bufs=2) as xgp, \
                 tc.tile_pool(name="gnsb", bufs=2) as gnp, \
                 tc.tile_pool(name="hn", bufs=1) as hnp, \
                 tc.tile_pool(name="wqkv", bufs=1) as wqkvp, \
                 tc.tile_pool(name="gnps", bufs=2, space="PSUM") as gnps, \
                 tc.tile_pool(name="projps", bufs=3, space="PSUM") as pjp:

                wq_sb = [wqkvp.tile([P, C], BF16, tag=f"wq{t}", name=f"wq{t}") for t in range(CT)]
                wk_sb = [wqkvp.tile([P, C], BF16, tag=f"wk{t}", name=f"wk{t}") for t in range(CT)]
                wv_sb = [wqkvp.tile([P, C], BF16, tag=f"wv{t}", name=f"wv{t}") for t in range(CT)]
                wq_t = wq_h.rearrange("(t p) o -> t p o", p=P)
                wk_t = wk_h.rearrange("(t p) o -> t p o", p=P)
                wv_t = wv_h.rearrange("(t p) o -> t p o", p=P)

                hn_bf = [hnp.tile([P, N], BF16, tag=f"hn{t}", name=f"hn{t}") for t in range(CT)]

                for ct in range(CT):
                    x_sb = xgp.tile([P, N], F32, tag="x")
                    # chunked load + per-chunk stats so stats start on the
                    # first chunk instead of after the full 2MB tile
                    stats = gnp.tile([P, 8, 6], F32, tag="stats")
                    for s in range(8):
                        nc.sync.dma_start(
                            out=x_sb[:, s * 512:(s + 1) * 512],
                            in_=x_t[ct][:, s * 512:(s + 1) * 512],
                        )
                        nc.vector.bn_stats(
                            out=stats[:, s, :], in_=x_sb[:, s * 512:(s + 1) * 512]
                        )
                    mv = gnp.tile([P, 2], F32, tag="mv")
                    nc.vector.bn_aggr(out=mv[:], in_=stats[:])
                    # cstat = [mean, E[x^2]] per channel
                    cstat = gnp.tile([P, 2], F32, tag="cstat")
                    nc.vector.tensor_copy(cstat[:, 0:1], mv[:, 0:1])
                    nc.vector.tensor_mul(cstat[:, 1:2], mv[:, 0:1], mv[:, 0:1])
                    nc.vector.tensor_add(cstat[:, 1:2], cstat[:, 1:2], mv[:, 1:2])
                    # group-average then broadcast back to channels (PE)
                    psA = gnps.tile([NG, 2], F32, tag="gn")
                    nc.tensor.matmul(psA[:], lhsT=g8_sb[:], rhs=cstat[:],
                                     start=True, stop=True)
                    gt = gnp.tile([NG, 2], F32, tag="gt")
                    nc.vector.tensor_copy(gt[:], psA[:])
                    psB = gnps.tile([P, 2], F32, tag="gn")
                    nc.tensor.matmul(psB[:], lhsT=g8T_sb[:], rhs=gt[:],
                                     start=True, stop=True)
                    gstat = gnp.tile([P, 2], F32, tag="gstat")
                    nc.vector.tensor_copy(gstat[:], psB[:])
                    # a = gamma * rsqrt(gvar+eps); d = beta - gmean * a
                    vtmp = gnp.tile([P, 1], F32, tag="vtmp")
                    nc.vector.tensor_mul(vtmp[:], gstat[:, 0:1], gstat[:, 0:1])
                    nc.vector.tensor_tensor(
                        out=vtmp[:], in0=gstat[:, 1:2], in1=vtmp[:],
                        op=mybir.AluOpType.subtract,
                    )
                    nc.scalar.activation(
                        out=vtmp[:], in_=vtmp[:],
                        func=mybir.ActivationFunctionType.Sqrt,
                        bias=eps_t[:], scale=1.0,
                    )
                    rstd = gnp.tile([P, 1], F32, tag="rstd")
                    nc.vector.reciprocal(out=rstd[:], in_=vtmp[:])
                    a_t = gnp.tile([P, 1], F32, tag="a_t")
                    nc.vector.tensor_mul(a_t[:], rstd[:], gam_sb[:, ct:ct + 1])
                    d_t = gnp.tile([P, 1], F32, tag="d_t")
                    nc.vector.tensor_mul(d_t[:], gstat[:, 0:1], a_t[:])
                    nc.vector.tensor_tensor(
                        out=d_t[:], in0=bet_sb[:, ct:ct + 1], in1=d_t[:],
                        op=mybir.AluOpType.subtract,
                    )
                    for s in range(8):
                        nc.scalar.activation(
                            out=hn_bf[ct][:, s * 512:(s + 1) * 512],
                            in_=x_sb[:, s * 512:(s + 1) * 512],
                            func=mybir.ActivationFunctionType.Identity,
                            scale=a_t[:], bias=d_t[:],
                        )


                # deferred weight loads (after x so groupnorm owns DMA at t=0)
                for t in range(CT):
                    nc.sync.dma_start(out=wq_sb[t][:], in_=wq_t[t])
                    nc.sync.dma_start(out=wk_sb[t][:], in_=wk_t[t])
                    nc.sync.dma_start(out=wv_sb[t][:], in_=wv_t[t])
                    nc.sync.dma_start(out=wo_sb[t][:], in_=wo_t[t])
                nc.sync.dma_start(
                    out=bv_bc[:],
                    in_=bass.AP(tensor=bv_h.tensor, offset=bv_h.offset, ap=[[0, P], [1, C]]),
                )

                # =========== Phase B: projections ===========
                # q[ct][c, i] (2048 query tokens), k[ct][c, j] (all 4096)
                for co in range(CT):
                    for isl in range(NSTRIP):
                        ps = pjp.tile([P, 512], F32, tag="pj")
                        for t in range(CT):
                            nc.tensor.matmul(
                                ps[:],
                                lhsT=wq_sb[t][:, co * P:(co + 1) * P],
                                rhs=hn_bf[t][:, isl * 512:(isl + 1) * 512],
                                start=(t == 0), stop=(t == CT - 1),
                            )
                        nc.vector.tensor_scalar_add(
                            out=q_bf[co][:, isl * 512:(isl + 1) * 512],
                            in0=ps[:], scalar1=bq_sb[:, co:co + 1],
                        )
                    for jsl in range(N // 512):
                        ps = pjp.tile([P, 512], F32, tag="pj")
                        for t in range(CT):
                            nc.tensor.matmul(
                                ps[:],
                                lhsT=wk_sb[t][:, co * P:(co + 1) * P],
                                rhs=hn_bf[t][:, jsl * 512:(jsl + 1) * 512],
                                start=(t == 0), stop=(t == CT - 1),
                            )
                        nc.vector.tensor_scalar_add(
                            out=k_bf[co][:, jsl * 512:(jsl + 1) * 512],
                            in0=ps[:], scalar1=bk_sb[:, co:co + 1],
                        )
                # v[jt][j, c] (token-major: one matmul per 128-token tile)
                for jt in range(JT):
                    ps = mmp.tile([P, 512], F32, tag="mm")
                    for t in range(CT):
                        nc.tensor.matmul(
                            ps[:],
                            lhsT=hn_bf[t][:, jt * P:(jt + 1) * P],
                            rhs=wv_sb[t][:],
                            start=(t == 0), stop=(t == CT - 1),
                        )
                    nc.vector.tensor_tensor(
                        out=v_bf[jt][:], in0=ps[:], in1=bv_bc[:],
                        op=mybir.AluOpType.add,
                    )


            with tc.tile_pool(name="hacc", bufs=4, space="PSUM") as hp, \
                 tc.tile_pool(name="lps", bufs=1, space="PSUM") as lp, \
                 tc.tile_pool(name="attn", bufs=1) as ap_, \
                 tc.tile_pool(name="lsb", bufs=2) as lsp, \
                 tc.tile_pool(name="xres", bufs=3) as xrp, \
                 tc.tile_pool(name="outt", bufs=3) as otp:

                # =========== Phase C: attention, software-pipelined strips ===========
                pT = [ap_.tile([P, 512], BF16, tag=f"pT{j}", name=f"pT{j}") for j in range(JT)]

                def emit_strip_core(st):
                    """scores -> exp -> colsums -> l roundtrip -> h matmuls -> h evac.
                    Returns the strip's h_bf tiles (normalized, bf16)."""
                    i0 = st * 512
                    for jt in range(JT):
                        ps = mmp.tile([P, 512], F32, tag="mm", name=f"s{st}_{jt}")
                        for t in range(CT):
                            nc.tensor.matmul(
                                ps[:],
                                lhsT=k_bf[t][:, jt * P:(jt + 1) * P],
                                rhs=q_bf[t][:, i0:i0 + 512],
                                start=(t == 0), stop=(t == CT - 1),
                            )
                        nc.scalar.activation(
                            out=pT[jt][:], in_=ps[:],
                            func=mybir.ActivationFunctionType.Exp,
                            scale=SCALE,
                        )
                    acc = lsp.tile([P, 512], F32, tag="lacc", name=f"lacc{st}")
                    nc.vector.tensor_tensor(
                        out=acc[:], in0=pT[0][:], in1=pT[1][:],
                        op=mybir.AluOpType.add,
                    )
                    for jt in range(2, JT):
                        nc.vector.tensor_tensor(
                            out=acc[:], in0=acc[:], in1=pT[jt][:],
                            op=mybir.AluOpType.add,
                        )
                    psl = lp.tile([1, 512], F32, tag="l", name=f"l{st}")
                    nc.tensor.matmul(
                        psl[:], lhsT=ones_f32[:], rhs=acc[:],
                        start=True, stop=True,
                    )
                    # 1/l on the single-partition row, then broadcast to all
                    # 128 partitions with a K=1 ones-matmul (all on-chip)
                    rl1 = lsp.tile([1, 512], F32, tag="rl1")
                    nc.vector.reciprocal(out=rl1[:], in_=psl[:])
                    psb = mmp.tile([P, 512], F32, tag="mm", name=f"rlbps{st}")
                    nc.tensor.matmul(psb[:], lhsT=ones1[:], rhs=rl1[:],
                                     start=True, stop=True)
                    rlb = lsp.tile([P, 512], F32, tag="rlb", name=f"rlb{st}")
                    nc.vector.tensor_copy(rlb[:], psb[:])
                    # h^T[c, i] = sum_j v[j, c] p[j, i] -- direct hT, no transposes
                    hps = [hp.tile([P, 512], F32, tag="h", name=f"hps{st}_{i}")
                           for i in range(CT)]
                    for jt in range(JT):
                        for cb in range(CT):
                            nc.tensor.matmul(
                                hps[cb][:],
                                lhsT=v_bf[jt][:, cb * P:(cb + 1) * P],
                                rhs=pT[jt][:],
                                start=(jt == 0), stop=(jt == JT - 1),
                            )
                    # normalize + evacuate straight into hT (bf16)
                    for cb in range(CT):
                        nc.vector.tensor_mul(
                            hT_bf[cb][:, i0:i0 + 512], hps[cb][:], rlb[:]
                        )
                    return None

                def emit_strip_tail(st, h_bfs):
                    """output projection + residual for one strip."""
                    i0 = st * 512
                    for co in range(CT):
                        ps = mmp.tile([P, 512], F32, tag="mm",
                                      name=f"op{st}_{co}")
                        for t in range(CT):
                            nc.tensor.matmul(
                                ps[:],
                                lhsT=wo_sb[t][:, co * P:(co + 1) * P],
                                rhs=hT_bf[t][:, i0:i0 + 512],
                                start=(t == 0), stop=(t == CT - 1),
                            )
                        xr = xrp.tile([P, 512], F32, tag="xr")
                        nc.sync.dma_start(
                            out=xr[:], in_=x_t[co][:, i0:i0 + 512]
                        )
                        ot = otp.tile([P, 512], F32, tag="ot")
                        nc.vector.tensor_scalar_add(
                            out=ot[:], in0=ps[:], scalar1=bo_sb[:, co:co + 1]
                        )
                        nc.vector.tensor_tensor(
                            out=ot[:], in0=ot[:], in1=xr[:],
                            op=mybir.AluOpType.add,
                        )
                        nc.sync.dma_start(
                            out=out_t[co][:, i0:i0 + 512], in_=ot[:]
                        )

                prev = None
                for st in range(NSTRIP):
                    h_bfs = emit_strip_core(st)
                    if prev is not None:
                        emit_strip_tail(prev[0], prev[1])
                    prev = (st, h_bfs)
                emit_strip_tail(prev[0], prev[1])

    nc.finalize()
    return nc


def kernel(**inputs):
    if "nc" not in _CACHE:
        _CACHE["nc"] = build_bass()
    nc = _CACHE["nc"]

    x = np.ascontiguousarray(np.asarray(inputs["x"], dtype=np.float32))
    B = x.shape[0]
    xf = x.reshape(B, C, N)

    def bfT(w):
        return np.ascontiguousarray(
            np.asarray(w, dtype=np.float32).T.astype(ml_dtypes.bfloat16)
        )

    shared = {
        "wqT": bfT(inputs["wq"]), "wkT": bfT(inputs["wk"]),
        "wvT": bfT(inputs["wv"]), "woT": bfT(inputs["wo"]),
        "bq": np.ascontiguousarray(np.asarray(inputs["bq"], np.float32)),
        "bk": np.ascontiguousarray(np.asarray(inputs["bk"], np.float32)),
        "bv": np.ascontiguousarray(np.asarray(inputs["bv"], np.float32)),
        "bo": np.ascontiguousarray(np.asarray(inputs["bo"], np.float32)),
        "gam": np.ascontiguousarray(np.asarray(inputs["norm_g"], np.float32)),
        "bet": np.ascontiguousarray(np.asarray(inputs["norm_b"], np.float32)),
    }

    in_maps = []
    for core in range(2 * B):
        b, half = core // 2, core % 2
        xb = xf[b]
        if half:
            xb = np.concatenate([xb[:, NQ:], xb[:, :NQ]], axis=1)
        in_maps.append({"x": np.ascontiguousarray(xb), **shared})

    import os
    trace = bool(os.environ.get("BASS_KERNEL_TRACE"))
    res = run_bass_kernel_spmd(
        nc, in_maps, core_ids=list(range(2 * B)), trace=trace,
        trace_cores=list(range(2 * B)) if trace else None,
    )
    _CACHE["last_results"] = res

    out = np.empty((B, C, N), np.float32)
    for core in range(2 * B):
        b, half = core // 2, core % 2
        out[b][:, half * NQ:(half + 1) * NQ] = res.results[core]["out"]
    return out.reshape(B, C, 64, 64)



# revision 36
# speedup vs baseline: 1.3492x; 1.3492x over previous
"""Trainium2 Bass kernel for nn_AttnBlock (GroupNorm + single-head 4096-token
attention + residual), sharded over 8 NeuronCores.

Sharding: data-parallel over batch B=4, sequence-parallel x2 over the 4096
query tokens -> 8 shards (token axis rolled on host for the second half so a
single SPMD NEFF serves all cores).

Compute strategy: fp8(e4m3) matmuls throughout. DoubleRow mode (0.5
cyc/row) is used where its 64-partition psum output is affordable;
plain-fp8 (1 cyc/row, full 128-partition psum) is mixed in to balance the
PE against the (ACT+DVE) evacuation/exp capacity:
  - scores: 27 of 32 key tiles per strip DoubleRow (2 exps of [64,512]
    with partition-offset pT writes), 5 plain (1 exp of [128,512]).
  - q/k/v projections: mostly DoubleRow, a slice plain; k-evacs on ACT,
    q/v-evacs on DVE, emission interleaved so both engines run.
  - p@v: DoubleRow; channels 0-255 accumulate in 4 psum banks during the
    strip (lagged 4 units behind scores so trailing v-evacs don't stall),
    channels 256-511 run as 4 16-matmul chunks through the scores pool
    during the NEXT strip (pT is double-buffered per strip, no WAR),
    each chunk evacuated immediately.
  - softmax denominator: fused ones-row DoubleRow matmul; 1/l applied at
    h-evacuation via a bf16 ones-matmul broadcast; the 1/l chain is
    emitted after the next strip's first unit so it never gates scores.
  - out-proj: plain fp8, evacuation fuses bias + residual in one DVE op.
GroupNorm's affine is folded into the q/k/v weights on device (w' = w*a,
bias' = b + w@d via tiny DoubleRow matmuls); per-chunk stats/fold math is
interleaved into the x DMA stream so the PE starts ~13us in. The v bias
folds into the out-proj bias (bo' = bo + wo@bv) so v evacuations are pure
copies; x is converted to fp8 once, overlapped with the x DMA (ACT/Pool).

Self-contained: hardcodes all shapes; only needs the concourse runtime.
"""

import numpy as np
import ml_dtypes

import concourse.bass as bass
import concourse.bacc as bacc
import concourse.tile as tile
from concourse import mybir
from concourse.bass_utils import run_bass_kernel_spmd

P = 128
C = 512
N = 4096
NQ = 2048
CT = 4                  # 128-channel chunks
U = 2                   # 256-channel DoubleRow pairs
JT = 32                 # key-token tiles of 128
M16 = 16                # key-token pair blocks of 256
NSTRIP = 4              # query strips of 512
HLAG = 4                # units of lag for the h accumulation
GS = 16
NG = P // GS
EPS = 1e-6
SCALE = float(C) ** -0.5
LN16 = 2.772588722239781
F32 = mybir.dt.float32
BF16 = mybir.dt.bfloat16
F8 = mybir.dt.float8e4
DR = mybir.MatmulPerfMode.DoubleRow
ADD = mybir.AluOpType.add
SUB = mybir.AluOpType.subtract
IDENT = mybir.ActivationFunctionType.Identity
EXP = mybir.ActivationFunctionType.Exp
SQRT = mybir.ActivationFunctionType.Sqrt

_CACHE = {}


def build_bass():
    nc = bacc.Bacc(None, target_bir_lowering=False)

    x_h = nc.dram_tensor("xbf", [C, N], BF16, kind="ExternalInput")[:]
    # xdr[u, p, i, n] = x[u*256 + i*128 + p, n] in fp8 (DoubleRow layout)
    xdr_h = nc.dram_tensor("xdr", [U, P, 2, N], F8, kind="ExternalInput")[:]
    # wdr[wi, u, p, i, o] = w_wi[o, u*256 + i*128 + p]; wi order q,k,v,o
    w_h = nc.dram_tensor("wdr", [4, U, P, 2, C], BF16, kind="ExternalInput")[:]
    bq_h = nc.dram_tensor("bq", [C], F32, kind="ExternalInput")[:]
    bk_h = nc.dram_tensor("bk", [C], F32, kind="ExternalInput")[:]
    bv_h = nc.dram_tensor("bv", [C], F32, kind="ExternalInput")[:]
    bo_h = nc.dram_tensor("bo", [C], F32, kind="ExternalInput")[:]
    gam_h = nc.dram_tensor("gam", [C], F32, kind="ExternalInput")[:]
    bet_h = nc.dram_tensor("bet", [C], F32, kind="ExternalInput")[:]
    out_h = nc.dram_tensor("out", [C, NQ], F32, kind="ExternalOutput")[:]

    g8_np = np.zeros((P, NG), np.float32)
    g8T_np = np.zeros((NG, P), np.float32)
    for c in range(P):
        g8_np[c, c // GS] = 1.0 / GS
        g8T_np[c // GS, c] = 1.0
    g8_h = nc.inline_tensor(g8_np, name="g8")[:]
    g8T_h = nc.inline_tensor(g8T_np, name="g8T")[:]

    x_t = x_h.rearrange("(t p) n -> t p n", p=P)
    out_t = out_h.rearrange("(t p) n -> t p n", p=P)

    def col4(ap1d):
        return bass.AP(tensor=ap1d.tensor, offset=ap1d.offset, ap=[[1, P], [P, CT]])

    def col8(ap1d):
        return bass.AP(tensor=ap1d.tensor, offset=ap1d.offset, ap=[[1, 64], [64, 8]])

    with tile.TileContext(nc) as tc:
        with tc.tile_pool(name="consts", bufs=1) as cp, \
             tc.tile_pool(name="xbf", bufs=1) as xbp, \
             tc.tile_pool(name="xf8", bufs=1) as x8p, \
             tc.tile_pool(name="wf8", bufs=1) as w8p, \
             tc.tile_pool(name="qkv", bufs=1) as qkvp, \
             tc.tile_pool(name="hT", bufs=1) as hTp, \
             tc.tile_pool(name="pT", bufs=1) as pTp:

            # ---- constants ----
            ones8 = cp.tile([P, 2, 64], F8, tag="ones8")
            nc.vector.memset(ones8[:], 1.0)
            eps_t = cp.tile([P, 1], F32, tag="eps")
            nc.vector.memset(eps_t[:], EPS)
            mln16 = cp.tile([P, 1], F32, tag="mln16")
            nc.vector.memset(mln16[:], -LN16)
            g8_sb = cp.tile([P, NG], F32, tag="g8")
            nc.sync.dma_start(out=g8_sb[:], in_=g8_h)
            g8T_sb = cp.tile([NG, P], F32, tag="g8T")
            nc.sync.dma_start(out=g8T_sb[:], in_=g8T_h)
            gam_sb = cp.tile([P, CT], F32, tag="gam")
            bet_sb = cp.tile([P, CT], F32, tag="bet")
            bo4 = cp.tile([P, CT], F32, tag="bo4")
            bo4p = cp.tile([P, CT], F32, tag="bo4p")
            bv4 = cp.tile([P, CT], F32, tag="bv4")
            bv8c = cp.tile([P, CT], F8, tag="bv8c")
            bq8_0 = cp.tile([64, 8], F32, tag="bq8_0")
            bk8_0 = cp.tile([64, 8], F32, tag="bk8_0")
            bq8 = cp.tile([64, 8], F32, tag="bq8")
            bk8 = cp.tile([64, 8], F32, tag="bk8")
            bkc0 = cp.tile([P, CT], F32, tag="bkc0")
            bk4p = cp.tile([P, CT], F32, tag="bk4p")
            A4 = cp.tile([P, CT], F32, tag="A4")
            D4 = cp.tile([P, CT], F32, tag="D4")
            RA = cp.tile([P, CT], F32, tag="RA")
            DRA = cp.tile([P, CT], F32, tag="DRA")
            dfa8 = cp.tile([P, 2, U], F8, tag="dfa8")

            # ---- persistent activations ----
            x_bf = [xbp.tile([P, N], BF16, tag=f"x{t}", name=f"x{t}")
                    for t in range(CT)]
            xf8 = [x8p.tile([P, 2, N], F8, tag=f"x8{u}", name=f"x8{u}")
                   for u in range(U)]
            wf8 = [[w8p.tile([P, 2, C], F8, tag=f"w8_{w}{u}", name=f"w8_{w}{u}")
                    for u in range(U)] for w in range(3)]
            wo8p = [w8p.tile([P, C], F8, tag=f"wo8_{t}", name=f"wo8_{t}")
                    for t in range(CT)]
            q2 = [qkvp.tile([P, 2, NQ], F8, tag=f"q{u}", name=f"q{u}")
                  for u in range(U)]
            k2 = [qkvp.tile([P, 2, N], F8, tag=f"k{u}", name=f"k{u}")
                  for u in range(U)]
            v2 = [qkvp.tile([P, 2, C], F8, tag=f"v{m}", name=f"v{m}")
                  for m in range(M16)]
            hT2 = [hTp.tile([P, 2, NQ], F8, tag=f"hT{u}", name=f"hT{u}")
                   for u in range(U)]
            pT2 = [[pTp.tile([P, 2, 512], F8, tag=f"pT{b}_{m}",
                             name=f"pT{b}_{m}") for m in range(M16)]
                   for b in range(2)]

            # =========== Phase A/B ===========
            with tc.tile_pool(name="wbf", bufs=1) as wbp, \
                 tc.tile_pool(name="gn", bufs=2) as gnp, \
                 tc.tile_pool(name="gnps", bufs=2, space="PSUM") as gnps, \
                 tc.tile_pool(name="pj", bufs=6, space="PSUM") as pjp:

                w_bf = [[wbp.tile([P, 2, C], BF16, tag=f"wb{w}{u}",
                                  name=f"wb{w}{u}")
                         for u in range(U)] for w in range(4)]

                # x (fp8, host-formatted) first, then wk + gn affine vectors
                for u in range(U):
                    nc.sync.dma_start(out=xf8[u][:], in_=xdr_h[u])
                for u in range(U):
                    nc.sync.dma_start(out=w_bf[1][u][:], in_=w_h[1][u])
                nc.sync.dma_start(out=gam_sb[:], in_=col4(gam_h))
                nc.sync.dma_start(out=bet_sb[:], in_=col4(bet_h))

                stats = gnp.tile([P, CT, 8, 6], F32, tag="stats")

                def gn_math(ct):
                    mv = gnp.tile([P, 2], F32, tag="mv", name=f"mv{ct}")
                    nc.vector.bn_aggr(out=mv[:], in_=stats[:, ct, :, :])
                    cstat = gnp.tile([P, 2], F32, tag="cstat", name=f"cs{ct}")
                    nc.vector.tensor_copy(cstat[:, 0:1], mv[:, 0:1])
                    nc.vector.tensor_mul(cstat[:, 1:2], mv[:, 0:1], mv[:, 0:1])
                    nc.vector.tensor_add(cstat[:, 1:2], cstat[:, 1:2], mv[:, 1:2])
                    psA = gnps.tile([NG, 2], F32, tag="gn", name=f"gA{ct}")
                    nc.tensor.matmul(psA[:], lhsT=g8_sb[:], rhs=cstat[:],
                                     start=True, stop=True)
                    gt = gnp.tile([NG, 2], F32, tag="gt", name=f"gt{ct}")
                    nc.vector.tensor_copy(gt[:], psA[:])
                    psB = gnps.tile([P, 2], F32, tag="gn", name=f"gB{ct}")
                    nc.tensor.matmul(psB[:], lhsT=g8T_sb[:], rhs=gt[:],
                                     start=True, stop=True)
                    gstat = gnp.tile([P, 2], F32, tag="gstat", name=f"gs{ct}")
                    nc.vector.tensor_copy(gstat[:], psB[:])
                    vtmp = gnp.tile([P, 1], F32, tag="vtmp", name=f"vt{ct}")
                    nc.vector.tensor_mul(vtmp[:], gstat[:, 0:1], gstat[:, 0:1])
                    nc.vector.tensor_tensor(out=vtmp[:], in0=gstat[:, 1:2],
                                            in1=vtmp[:], op=SUB)
                    nc.scalar.activation(out=vtmp[:], in_=vtmp[:], func=SQRT,
                                         bias=eps_t[:], scale=1.0)
                    rstd = gnp.tile([P, 1], F32, tag="rstd", name=f"rs{ct}")
                    nc.vector.reciprocal(out=rstd[:], in_=vtmp[:])
                    nc.vector.tensor_mul(A4[:, ct:ct + 1], rstd[:],
                                         gam_sb[:, ct:ct + 1])
                    dt_ = gnp.tile([P, 1], F32, tag="dt", name=f"dt{ct}")
                    nc.vector.tensor_mul(dt_[:], gstat[:, 0:1], A4[:, ct:ct + 1])
                    nc.vector.tensor_tensor(out=D4[:, ct:ct + 1],
                                            in0=bet_sb[:, ct:ct + 1],
                                            in1=dt_[:], op=SUB)

                # stats straight off the fp8 x; gn math + wk-scale per chunk
                for ct in range(CT):
                    u, i = ct // 2, ct % 2
                    for s8 in range(8):
                        nc.vector.bn_stats(
                            out=stats[:, ct, s8, :],
                            in_=xf8[u][:, i, s8 * 512:(s8 + 1) * 512],
                        )
                    gn_math(ct)
                    nc.vector.tensor_scalar_mul(wf8[1][u][:, i, :],
                                                w_bf[1][u][:, i, :],
                                                A4[:, ct:ct + 1])

                # remaining weights, residual x (bf16), small vectors
                for w in (2, 0, 3):
                    for u in range(U):
                        nc.sync.dma_start(out=w_bf[w][u][:], in_=w_h[w][u])
                nc.sync.dma_start(out=bo4[:], in_=col4(bo_h))
                nc.sync.dma_start(out=bv4[:], in_=col4(bv_h))
                nc.sync.dma_start(out=bq8_0[:], in_=col8(bq_h))
                nc.sync.dma_start(out=bk8_0[:], in_=col8(bk_h))
                nc.sync.dma_start(out=bkc0[:], in_=col4(bk_h))
                for ct in range(CT):
                    nc.sync.dma_start(out=x_bf[ct][:], in_=x_t[ct])

                nc.vector.reciprocal(out=RA[:], in_=A4[:])
                nc.vector.tensor_mul(DRA[:], D4[:], RA[:])
                dsrc = DRA[:]
                nc.vector.tensor_copy(
                    dfa8[:],
                    bass.AP(tensor=dsrc.tensor, offset=dsrc.offset,
                            ap=[dsrc.ap[0], [1, 2], [2, U]]),
                )
                nc.vector.tensor_copy(bv8c[:], bv4[:])

                # wq / wv scaling and wo conversion on Pool
                for cc in range(CT):
                    u, i = cc // 2, cc % 2
                    acol = A4[:, cc:cc + 1]
                    nc.gpsimd.tensor_scalar_mul(wf8[0][u][:, i, :],
                                                w_bf[0][u][:, i, :], acol)
                    nc.gpsimd.tensor_scalar_mul(wf8[2][u][:, i, :],
                                                w_bf[2][u][:, i, :], acol)
                for cc in range(CT):
                    nc.gpsimd.tensor_copy(wo8p[cc][:],
                                          w_bf[3][cc // 2][:, cc % 2, :])

                # bias folds b' = b + w @ d in [64,8] block-column layout
                def bias_fold8(w, b0, bout, name):
                    for ob in range(8):
                        pf = gnps.tile([64, 1], F32, tag="gn",
                                       name=f"bf{name}{ob}")
                        for u in range(U):
                            nc.tensor.matmul(
                                pf[:],
                                lhsT=wf8[w][u][:, :, ob * 64:(ob + 1) * 64],
                                rhs=dfa8[:, :, u:u + 1],
                                start=(u == 0), stop=(u == U - 1),
                                perf_mode=DR)
                        nc.vector.tensor_tensor(out=bout[:, ob:ob + 1],
                                                in0=pf[:],
                                                in1=b0[:, ob:ob + 1], op=ADD)

                bias_fold8(1, bk8_0, bk8, "k")

                # plain-k blocks need the folded bias in [128,4] col layout
                for b in range(CT):
                    pf4 = gnps.tile([P, 1], F32, tag="gn", name=f"bk4_{b}")
                    for cc in range(CT):
                        nc.tensor.matmul(
                            pf4[:],
                            lhsT=wf8[1][cc // 2][:, cc % 2, b * P:(b + 1) * P],
                            rhs=dfa8[:, cc % 2, cc // 2:cc // 2 + 1],
                            start=(cc == 0), stop=(cc == CT - 1))
                    nc.vector.tensor_tensor(out=bk4p[:, b:b + 1], in0=pf4[:],
                                            in1=bkc0[:, b:b + 1], op=ADD)

                bias_fold8(0, bq8_0, bq8, "q")

                # bo' = bo + wo @ bv (plain fp8)
                for ct_o in range(CT):
                    pbo = gnps.tile([P, 1], F32, tag="gn", name=f"bo{ct_o}")
                    for cc in range(CT):
                        nc.tensor.matmul(
                            pbo[:],
                            lhsT=wo8p[cc][:, ct_o * P:(ct_o + 1) * P],
                            rhs=bv8c[:, cc:cc + 1],
                            start=(cc == 0), stop=(cc == CT - 1))
                    nc.vector.tensor_tensor(out=bo4p[:, ct_o:ct_o + 1],
                                            in0=pbo[:],
                                            in1=bo4[:, ct_o:ct_o + 1], op=ADD)

                # ---- k / v projections, interleaved (k evac ACT, v DVE) ----
                def emit_k_jsl(jsl):
                    tsl = slice(jsl * 512, (jsl + 1) * 512)
                    if jsl >= 6:        # plain fp8: full-width psum
                        for b in range(CT):
                            psk = pjp.tile([P, 512], F32, tag="pj",
                                           name=f"k{jsl}_{b}")
                            for cc in range(CT):
                                nc.tensor.matmul(
                                    psk[:],
                                    lhsT=wf8[1][cc // 2][:, cc % 2,
                                                         b * P:(b + 1) * P],
                                    rhs=xf8[cc // 2][:, cc % 2, tsl],
                                    start=(cc == 0), stop=(cc == CT - 1))
                            nc.scalar.activation(
                                out=k2[b // 2][:, b % 2, tsl], in_=psk[:],
                                func=IDENT, scale=1.0,
                                bias=bk4p[:, b:b + 1])
                    else:               # DoubleRow
                        for ob in range(8):
                            psk = pjp.tile([P, 512], F32, tag="pj",
                                           name=f"k{jsl}_{ob}")
                            for u in range(U):
                                nc.tensor.matmul(
                                    psk[0:64, :],
                                    lhsT=wf8[1][u][:, :, ob * 64:(ob + 1) * 64],
                                    rhs=xf8[u][:, :, tsl],
                                    start=(u == 0), stop=(u == U - 1),
                                    perf_mode=DR)
                            po = (ob % 2) * 64
                            nc.scalar.activation(
                                out=k2[ob // 4][po:po + 64, (ob % 4) // 2, tsl],
                                in_=psk[0:64, :], func=IDENT,
                                scale=1.0, bias=bk8[:, ob:ob + 1])

                def emit_v_tb2(tb2):
                    m, i = tb2 // 2, tb2 % 2
                    if tb2 >= 22:       # plain fp8: full-width psum
                        psv = pjp.tile([P, 512], F32, tag="pj",
                                       name=f"vp{tb2}")
                        t0 = tb2 * P
                        for cc in range(CT):
                            nc.tensor.matmul(
                                psv[:],
                                lhsT=xf8[cc // 2][:, cc % 2, t0:t0 + P],
                                rhs=wf8[2][cc // 2][:, cc % 2, :],
                                start=(cc == 0), stop=(cc == CT - 1))
                        if tb2 % 2 == 0:
                            nc.scalar.mul(v2[m][:, i, :], psv[:], 1.0)
                        else:
                            nc.vector.tensor_copy(v2[m][:, i, :], psv[:])
                    else:               # DoubleRow, two 64-token halves
                        for hh in range(2):
                            psv = pjp.tile([P, 512], F32, tag="pj",
                                           name=f"v{tb2}_{hh}")
                            t0 = tb2 * P + hh * 64
                            for u in range(U):
                                nc.tensor.matmul(
                                    psv[0:64, :],
                                    lhsT=xf8[u][:, :, t0:t0 + 64],
                                    rhs=wf8[2][u][:],
                                    start=(u == 0), stop=(u == U - 1),
                                    perf_mode=DR)
                            po = hh * 64
                            nc.vector.tensor_copy(v2[m][po:po + 64, i, :],
                                                  psv[0:64, :])

                for g in range(8):
                    emit_k_jsl(g)
                    for j in range(4):
                        emit_v_tb2(4 * g + j)

                def emit_qproj(st, pool, tag, obs, on_act=False):
                    tsl = slice(st * 512, (st + 1) * 512)
                    for ob in obs:
                        psq = pool.tile([P, 512], F32, tag=tag,
                                        name=f"q{st}_{ob}")
                        for u in range(U):
                            nc.tensor.matmul(
                                psq[0:64, :],
                                lhsT=wf8[0][u][:, :, ob * 64:(ob + 1) * 64],
                                rhs=xf8[u][:, :, tsl],
                                start=(u == 0), stop=(u == U - 1),
                                perf_mode=DR)
                        po = (ob % 2) * 64
                        dst = q2[ob // 4][po:po + 64, (ob % 4) // 2, tsl]
                        if on_act:
                            nc.scalar.activation(out=dst, in_=psq[0:64, :],
                                                 func=IDENT, scale=1.0,
                                                 bias=bq8[:, ob:ob + 1])
                        else:
                            nc.vector.tensor_scalar_add(
                                out=dst, in0=psq[0:64, :],
                                scalar1=bq8[:, ob:ob + 1])

                emit_qproj(0, pjp, "pj", range(8), on_act=True)

            # =========== Phase C: attention strips ===========
            with tc.tile_pool(name="sc", bufs=7, space="PSUM") as scp, \
                 tc.tile_pool(name="lps", bufs=1, space="PSUM") as lp, \
                 tc.tile_pool(name="lsb", bufs=2) as lsp, \
                 tc.tile_pool(name="outt", bufs=3) as otp:

                rlb_of = {}
                psl_of = {}

                def emit_scores_unit(st, m, pT):
                    isl = slice(st * 512, (st + 1) * 512)
                    for jj in range(2):
                        jt = 2 * m + jj
                        if jt % 6 == 5:     # plain fp8 scores
                            ps = scp.tile([P, 512], F32, tag="sc",
                                          name=f"s{st}_{jt}")
                            j0 = jt * P
                            for cc in range(CT):
                                nc.tensor.matmul(
                                    ps[:],
                                    lhsT=k2[cc // 2][:, cc % 2, j0:j0 + P],
                                    rhs=q2[cc // 2][:, cc % 2, isl],
                                    start=(cc == 0), stop=(cc == CT - 1))
                            nc.scalar.activation(
                                out=pT[m][:, jj, :], in_=ps[:],
                                func=EXP, scale=SCALE, bias=mln16[:])
                        else:               # DoubleRow scores
                            for hh in range(2):
                                ps = scp.tile([P, 512], F32, tag="sc",
                                              name=f"s{st}_{jt}_{hh}")
                                j0 = jt * P + hh * 64
                                for u in range(U):
                                    nc.tensor.matmul(
                                        ps[0:64, :],
                                        lhsT=k2[u][:, :, j0:j0 + 64],
                                        rhs=q2[u][:, :, isl],
                                        start=(u == 0), stop=(u == U - 1),
                                        perf_mode=DR)
                                nc.scalar.activation(
                                    out=pT[m][hh * 64:(hh + 1) * 64, jj, :],
                                    in_=ps[0:64, :], func=EXP,
                                    scale=SCALE, bias=mln16[0:64, :])

                def emit_l_unit(st, m, pT):
                    # all-ones M=64 stationary: every psum row accumulates l,
                    # so the result is already broadcast across 64 partitions
                    nc.tensor.matmul(psl_of[st][0:64, :], lhsT=ones8[:],
                                     rhs=pT[m][:],
                                     start=(m == 0), stop=(m == M16 - 1),
                                     perf_mode=DR)

                def emit_rl_chain(st):
                    rlb = lsp.tile([64, 512], F32, tag="rlb", name=f"rlb{st}")
                    nc.vector.reciprocal(out=rlb[:], in_=psl_of[st][0:64, :])
                    rlb_of[st] = rlb

                def emit_h_chunk(sp, cb):
                    # h accumulation for channels cb*64.. of strip sp
                    i0 = sp * 512
                    pT = pT2[sp % 2]
                    psb_ = scp.tile([P, 512], F32, tag="sc",
                                    name=f"hB{sp}_{cb}")
                    for m in range(M16):
                        nc.tensor.matmul(
                            psb_[0:64, :],
                            lhsT=v2[m][:, :, cb * 64:(cb + 1) * 64],
                            rhs=pT[m][:],
                            start=(m == 0), stop=(m == M16 - 1),
                            perf_mode=DR)
                    po = (cb % 2) * 64
                    nc.vector.tensor_mul(
                        hT2[cb // 4][po:po + 64, (cb % 4) // 2, i0:i0 + 512],
                        psb_[0:64, :], rlb_of[sp][:])

                def emit_outproj(sp, blocks):
                    i0 = sp * 512
                    isl = slice(i0, i0 + 512)
                    for b in blocks:
                        pso = scp.tile([P, 512], F32, tag="sc",
                                       name=f"op{sp}_{b}")
                        for cc in range(CT):
                            nc.tensor.matmul(
                                pso[:],
                                lhsT=wo8p[cc][:, b * P:(b + 1) * P],
                                rhs=hT2[cc // 2][:, cc % 2, isl],
                                start=(cc == 0), stop=(cc == CT - 1))
                        ot = otp.tile([P, 512], F32, tag="ot",
                                      name=f"ot{sp}_{b}")
                        nc.vector.scalar_tensor_tensor(
                            out=ot[:], in0=pso[:], scalar=bo4p[:, b:b + 1],
                            in1=x_bf[b][:, isl], op0=ADD, op1=ADD)
                        nc.sync.dma_start(out=out_t[b][:, isl], in_=ot[:])

                for st in range(NSTRIP):
                    pT = pT2[st % 2]
                    psl_of[st] = lp.tile([P, 512], F32, tag="l",
                                         name=f"l{st}")
                    for m in range(M16):
                        emit_scores_unit(st, m, pT)
                        if m >= 1:
                            emit_l_unit(st, m - 1, pT)
                        if st > 0:
                            if m == 1:
                                emit_rl_chain(st - 1)
                            elif 3 <= m <= 10:
                                emit_h_chunk(st - 1, m - 3)
                            elif m in (12, 13):
                                emit_outproj(st - 1, (2 * (m - 12),
                                                      2 * (m - 12) + 1))
                        if st < NSTRIP - 1 and m in (14, 15):
                            emit_qproj(st + 1, scp, "sc",
                                       range(4 * (m - 14), 4 * (m - 13)))
                    emit_l_unit(st, M16 - 1, pT)

                # tail: last strip's h chunks with the out-proj matmuls
                # interleaved (each cc half of hT2 becomes ready after two
                # chunks), then fused evacuation + store
                sp = NSTRIP - 1
                i0 = sp * 512
                isl = slice(i0, i0 + 512)
                emit_rl_chain(sp)
                op_ps = []
                for cb in range(8):
                    emit_h_chunk(sp, cb)
                    if cb % 2 == 1:
                        cc = cb // 2
                        for b in range(CT):
                            if cc == 0:
                                op_ps.append(scp.tile([P, 512], F32, tag="sc",
                                                      name=f"opt_{b}"))
                            nc.tensor.matmul(
                                op_ps[b][:],
                                lhsT=wo8p[cc][:, b * P:(b + 1) * P],
                                rhs=hT2[cc // 2][:, cc % 2, isl],
                                start=(cc == 0), stop=(cc == CT - 1))
                for b in range(CT):
                    ot = otp.tile([P, 512], F32, tag="ot", name=f"ott_{b}")
                    nc.vector.scalar_tensor_tensor(
                        out=ot[:], in0=op_ps[b][:], scalar=bo4p[:, b:b + 1],
                        in1=x_bf[b][:, isl], op0=ADD, op1=ADD)
                    nc.sync.dma_start(out=out_t[b][:, isl], in_=ot[:])

    nc.finalize()
    return nc


def kernel(**inputs):
    if "nc" not in _CACHE:
        _CACHE["nc"] = build_bass()
    nc = _CACHE["nc"]

    x = np.asarray(inputs["x"], dtype=np.float32)
    B = x.shape[0]
    xf = x.reshape(B, C, N)

    def to_dr(w):
        wT = np.asarray(w, dtype=np.float32).T        # [c, o]
        return wT.reshape(U, 2, P, C).transpose(0, 2, 1, 3)

    wdr = np.ascontiguousarray(
        np.stack([to_dr(inputs[k]) for k in ("wq", "wk", "wv", "wo")])
        .astype(ml_dtypes.bfloat16)
    )

    shared = {
        "wdr": wdr,
        "bq": np.ascontiguousarray(np.asarray(inputs["bq"], np.float32)),
        "bk": np.ascontiguousarray(np.asarray(inputs["bk"], np.float32)),
        "bv": np.ascontiguousarray(np.asarray(inputs["bv"], np.float32)),
        "bo": np.ascontiguousarray(np.asarray(inputs["bo"], np.float32)),
        "gam": np.ascontiguousarray(np.asarray(inputs["norm_g"], np.float32)),
        "bet": np.ascontiguousarray(np.asarray(inputs["norm_b"], np.float32)),
    }

    in_maps = []
    for core in range(2 * B):
        b, half = core // 2, core % 2
        xb = xf[b]
        if half:
            xb = np.concatenate([xb[:, NQ:], xb[:, :NQ]], axis=1)
        # fp8 copy in DoubleRow layout [u, p, i, n]
        xdr = np.ascontiguousarray(
            xb.reshape(U, 2, P, N).transpose(0, 2, 1, 3)
            .astype(ml_dtypes.float8_e4m3))
        in_maps.append(
            {"xbf": np.ascontiguousarray(xb.astype(ml_dtypes.bfloat16)),
             "xdr": xdr, **shared})

    import os
    trace = bool(os.environ.get("BASS_KERNEL_TRACE"))
    res = run_bass_kernel_spmd(
        nc, in_maps, core_ids=list(range(2 * B)), trace=trace,
        trace_cores=list(range(2 * B)) if trace else None,
    )
    _CACHE["last_results"] = res

    out = np.empty((B, C, N), np.float32)
    for core in range(2 * B):
        b, half = core // 2, core % 2
        out[b][:, half * NQ:(half + 1) * NQ] = res.results[core]["out"]
    return out.reshape(B, C, 64, 64)


# revision 49
# speedup vs baseline: 1.3856x; 1.0270x over previous
"""Trainium2 Bass kernel for nn_AttnBlock (GroupNorm + single-head 4096-token
attention + residual), sharded over 8 NeuronCores.

Sharding: data-parallel over batch B=4, sequence-parallel x2 over the 4096
query tokens -> 8 shards (token axis rolled on host for the second half so a
single SPMD NEFF serves all cores).

Compute strategy: fp8(e4m3) matmuls throughout. DoubleRow mode (0.5
cyc/row) is used where its 64-partition psum output is affordable;
plain-fp8 (1 cyc/row, full 128-partition psum) is mixed in to balance the
PE against the (ACT+DVE) evacuation/exp capacity:
  - scores: 27 of 32 key tiles per strip DoubleRow (2 exps of [64,512]
    with partition-offset pT writes), 5 plain (1 exp of [128,512]).
  - q/k/v projections: mostly DoubleRow, a slice plain; k-evacs on ACT,
    q/v-evacs on DVE, emission interleaved so both engines run.
  - p@v: DoubleRow; channels 0-255 accumulate in 4 psum banks during the
    strip (lagged 4 units behind scores so trailing v-evacs don't stall),
    channels 256-511 run as 4 16-matmul chunks through the scores pool
    during the NEXT strip (pT is double-buffered per strip, no WAR),
    each chunk evacuated immediately.
  - softmax denominator: fused ones-row DoubleRow matmul; 1/l applied at
    h-evacuation via a bf16 ones-matmul broadcast; the 1/l chain is
    emitted after the next strip's first unit so it never gates scores.
  - out-proj: plain fp8, evacuation fuses bias + residual in one DVE op.
GroupNorm's affine is folded into the q/k/v weights on device (w' = w*a,
bias' = b + w@d via tiny DoubleRow matmuls); per-chunk stats/fold math is
interleaved into the x DMA stream so the PE starts ~13us in. The v bias
folds into the out-proj bias (bo' = bo + wo@bv) so v evacuations are pure
copies; x is converted to fp8 once, overlapped with the x DMA (ACT/Pool).

Self-contained: hardcodes all shapes; only needs the concourse runtime.
"""

import numpy as np
import ml_dtypes

import concourse.bass as bass
import concourse.bacc as bacc
import concourse.tile as tile
from concourse import mybir
from concourse.bass_utils import run_bass_kernel_spmd

P = 128
C = 512
N = 4096
NQ = 2048
CT = 4                  # 128-channel chunks
U = 2                   # 256-channel DoubleRow pairs
JT = 32                 # key-token tiles of 128
M16 = 16                # key-token pair blocks of 256
NSTRIP = 4              # query strips of 512
HLAG = 4                # units of lag for the h accumulation
GS = 16
NG = P // GS
EPS = 1e-6
SCALE = float(C) ** -0.5
LN16 = 2.772588722239781
F32 = mybir.dt.float32
BF16 = mybir.dt.bfloat16
F8 = mybir.dt.float8e4
DR = mybir.MatmulPerfMode.DoubleRow
ADD = mybir.AluOpType.add
SUB = mybir.AluOpType.subtract
IDENT = mybir.ActivationFunctionType.Identity
EXP = mybir.ActivationFunctionType.Exp
SQRT = mybir.ActivationFunctionType.Sqrt

_CACHE = {}


def build_bass():
    nc = bacc.Bacc(None, target_bir_lowering=False)

    x_h = nc.dram_tensor("xbf", [C, N], BF16, kind="ExternalInput")[:]
    # xdr[u, p, i, n] = x[u*256 + i*128 + p, n] in fp8 (DoubleRow layout)
    xdr_h = nc.dram_tensor("xdr", [U, P, 2, N], F8, kind="ExternalInput")[:]
    # wdr[wi, u, p, i, o] = w_wi[o, u*256 + i*128 + p]; wi order q,k,v,o
    w_h = nc.dram_tensor("wdr", [4, U, P, 2, C], BF16, kind="ExternalInput")[:]
    bq_h = nc.dram_tensor("bq", [C], F32, kind="ExternalInput")[:]
    bk_h = nc.dram_tensor("bk", [C], F32, kind="ExternalInput")[:]
    bv_h = nc.dram_tensor("bv", [C], F32, kind="ExternalInput")[:]
    bo_h = nc.dram_tensor("bo", [C], F32, kind="ExternalInput")[:]
    gam_h = nc.dram_tensor("gam", [C], F32, kind="ExternalInput")[:]
    bet_h = nc.dram_tensor("bet", [C], F32, kind="ExternalInput")[:]
    out_h = nc.dram_tensor("out", [C, NQ], F32, kind="ExternalOutput")[:]

    g8_np = np.zeros((P, NG), np.float32)
    g8T_np = np.zeros((NG, P), np.float32)
    for c in range(P):
        g8_np[c, c // GS] = 1.0 / GS
        g8T_np[c // GS, c] = 1.0
    g8_h = nc.inline_tensor(g8_np, name="g8")[:]
    g8T_h = nc.inline_tensor(g8T_np, name="g8T")[:]

    x_t = x_h.rearrange("(t p) n -> t p n", p=P)
    out_t = out_h.rearrange("(t p) n -> t p n", p=P)

    def col4(ap1d):
        return bass.AP(tensor=ap1d.tensor, offset=ap1d.offset, ap=[[1, P], [P, CT]])

    def col8(ap1d):
        return bass.AP(tensor=ap1d.tensor, offset=ap1d.offset, ap=[[1, 64], [64, 8]])

    with tile.TileContext(nc) as tc:
        with tc.tile_pool(name="consts", bufs=1) as cp, \
             tc.tile_pool(name="xbf", bufs=1) as xbp, \
             tc.tile_pool(name="xf8", bufs=1) as x8p, \
             tc.tile_pool(name="wf8", bufs=1) as w8p, \
             tc.tile_pool(name="qkv", bufs=1) as qkvp, \
             tc.tile_pool(name="hT", bufs=1) as hTp, \
             tc.tile_pool(name="pT", bufs=1) as pTp:

            # ---- constants ----
            ones8 = cp.tile([P, 2, 64], F8, tag="ones8")
            nc.vector.memset(ones8[:], 1.0)
            eps_t = cp.tile([P, 1], F32, tag="eps")
            nc.vector.memset(eps_t[:], EPS)
            mln16 = cp.tile([P, 1], F32, tag="mln16")
            nc.vector.memset(mln16[:], -LN16)
            g8_sb = cp.tile([P, NG], F32, tag="g8")
            nc.sync.dma_start(out=g8_sb[:], in_=g8_h)
            g8T_sb = cp.tile([NG, P], F32, tag="g8T")
            nc.sync.dma_start(out=g8T_sb[:], in_=g8T_h)
            gam_sb = cp.tile([P, CT], F32, tag="gam")
            bet_sb = cp.tile([P, CT], F32, tag="bet")
            bo4 = cp.tile([P, CT], F32, tag="bo4")
            bo4p = cp.tile([P, CT], F32, tag="bo4p")
            bv4 = cp.tile([P, CT], F32, tag="bv4")
            bv8c = cp.tile([P, CT], F8, tag="bv8c")
            bq8_0 = cp.tile([64, 8], F32, tag="bq8_0")
            bk8_0 = cp.tile([64, 8], F32, tag="bk8_0")
            bq8 = cp.tile([64, 8], F32, tag="bq8")
            bk8 = cp.tile([64, 8], F32, tag="bk8")
            bkc0 = cp.tile([P, CT], F32, tag="bkc0")
            bk4p = cp.tile([P, CT], F32, tag="bk4p")
            A4 = cp.tile([P, CT], F32, tag="A4")
            D4 = cp.tile([P, CT], F32, tag="D4")
            RA = cp.tile([P, CT], F32, tag="RA")
            DRA = cp.tile([P, CT], F32, tag="DRA")
            dfa8 = cp.tile([P, 2, U], F8, tag="dfa8")

            # ---- persistent activations ----
            x_bf = [xbp.tile([P, N], BF16, tag=f"x{t}", name=f"x{t}")
                    for t in range(CT)]
            xf8 = [x8p.tile([P, 2, N], F8, tag=f"x8{u}", name=f"x8{u}")
                   for u in range(U)]
            wf8 = [[w8p.tile([P, 2, C], F8, tag=f"w8_{w}{u}", name=f"w8_{w}{u}")
                    for u in range(U)] for w in range(3)]
            wo8p = [w8p.tile([P, C], F8, tag=f"wo8_{t}", name=f"wo8_{t}")
                    for t in range(CT)]
            q2 = [qkvp.tile([P, 2, NQ], F8, tag=f"q{u}", name=f"q{u}")
                  for u in range(U)]
            k2 = [qkvp.tile([P, 2, N], F8, tag=f"k{u}", name=f"k{u}")
                  for u in range(U)]
            v2 = [qkvp.tile([P, 2, C], F8, tag=f"v{m}", name=f"v{m}")
                  for m in range(M16)]
            hT2 = [hTp.tile([P, 2, NQ], F8, tag=f"hT{u}", name=f"hT{u}")
                   for u in range(U)]
            pT2 = [[pTp.tile([P, 2, 512], F8, tag=f"pT{b}_{m}",
                             name=f"pT{b}_{m}") for m in range(M16)]
                   for b in range(2)]

            # =========== Phase A/B ===========
            with tc.tile_pool(name="wbf", bufs=1) as wbp, \
                 tc.tile_pool(name="gn", bufs=2) as gnp, \
                 tc.tile_pool(name="gnps", bufs=2, space="PSUM") as gnps, \
                 tc.tile_pool(name="pj", bufs=6, space="PSUM") as pjp:

                w_bf = [[wbp.tile([P, 2, C], BF16, tag=f"wb{w}{u}",
                                  name=f"wb{w}{u}")
                         for u in range(U)] for w in range(4)]

                # x (fp8, host-formatted) first, then wk + gn affine vectors
                for ct in range(CT):
                    u, i = ct // 2, ct % 2
                    nc.sync.dma_start(out=xf8[u][:, i, :], in_=xdr_h[u][:, i, :])
                for u in range(U):
                    nc.sync.dma_start(out=w_bf[1][u][:], in_=w_h[1][u])
                nc.sync.dma_start(out=gam_sb[:], in_=col4(gam_h))
                nc.sync.dma_start(out=bet_sb[:], in_=col4(bet_h))

                stats = gnp.tile([P, CT, 8, 6], F32, tag="stats")
                asums = gnp.tile([P, 2], F32, tag="asums")
                ascr = gnp.tile([P, N], F8, tag="ascr")

                def gn_math(ct):
                    cstat = gnp.tile([P, 2], F32, tag="cstat", name=f"cs{ct}")
                    if ct == CT - 1:
                        # ct3 stats arrive as [sum(x), sum(x^2)] from ACT
                        nc.vector.tensor_scalar_mul(cstat[:], asums[:],
                                                    1.0 / N)
                    else:
                        mv = gnp.tile([P, 2], F32, tag="mv", name=f"mv{ct}")
                        nc.vector.bn_aggr(out=mv[:], in_=stats[:, ct, :, :])
                        nc.vector.tensor_copy(cstat[:, 0:1], mv[:, 0:1])
                        nc.vector.tensor_mul(cstat[:, 1:2], mv[:, 0:1],
                                             mv[:, 0:1])
                        nc.vector.tensor_add(cstat[:, 1:2], cstat[:, 1:2],
                                             mv[:, 1:2])
                    psA = gnps.tile([NG, 2], F32, tag="gn", name=f"gA{ct}")
                    nc.tensor.matmul(psA[:], lhsT=g8_sb[:], rhs=cstat[:],
                                     start=True, stop=True)
                    gt = gnp.tile([NG, 2], F32, tag="gt", name=f"gt{ct}")
                    nc.vector.tensor_copy(gt[:], psA[:])
                    psB = gnps.tile([P, 2], F32, tag="gn", name=f"gB{ct}")
                    nc.tensor.matmul(psB[:], lhsT=g8T_sb[:], rhs=gt[:],
                                     start=True, stop=True)
                    gstat = gnp.tile([P, 2], F32, tag="gstat", name=f"gs{ct}")
                    nc.vector.tensor_copy(gstat[:], psB[:])
                    vtmp = gnp.tile([P, 1], F32, tag="vtmp", name=f"vt{ct}")
                    nc.vector.tensor_mul(vtmp[:], gstat[:, 0:1], gstat[:, 0:1])
                    nc.vector.tensor_tensor(out=vtmp[:], in0=gstat[:, 1:2],
                                            in1=vtmp[:], op=SUB)
                    nc.scalar.activation(out=vtmp[:], in_=vtmp[:], func=SQRT,
                                         bias=eps_t[:], scale=1.0)
                    rstd = gnp.tile([P, 1], F32, tag="rstd", name=f"rs{ct}")
                    nc.vector.reciprocal(out=rstd[:], in_=vtmp[:])
                    nc.vector.tensor_mul(A4[:, ct:ct + 1], rstd[:],
                                         gam_sb[:, ct:ct + 1])
                    dt_ = gnp.tile([P, 1], F32, tag="dt", name=f"dt{ct}")
                    nc.vector.tensor_mul(dt_[:], gstat[:, 0:1], A4[:, ct:ct + 1])
                    nc.vector.tensor_tensor(out=D4[:, ct:ct + 1],
                                            in0=bet_sb[:, ct:ct + 1],
                                            in1=dt_[:], op=SUB)

                # stats straight off the fp8 x; gn math + wk-scale per chunk.
                # ct3's sums run on the otherwise-idle ACT engine so the
                # DVE-serial stats chain is ~25% shorter.
                nc.scalar.activation(out=ascr[:], in_=xf8[1][:, 1, :],
                                     func=mybir.ActivationFunctionType.Copy,
                                     accum_out=asums[:, 0:1])
                nc.scalar.activation(out=ascr[:], in_=xf8[1][:, 1, :],
                                     func=mybir.ActivationFunctionType.Square,
                                     accum_out=asums[:, 1:2])
                for ct in range(CT):
                    u, i = ct // 2, ct % 2
                    if ct < CT - 1:
                        for s8 in range(8):
                            nc.vector.bn_stats(
                                out=stats[:, ct, s8, :],
                                in_=xf8[u][:, i, s8 * 512:(s8 + 1) * 512],
                            )
                    gn_math(ct)
                    nc.vector.tensor_scalar_mul(wf8[1][u][:, i, :],
                                                w_bf[1][u][:, i, :],
                                                A4[:, ct:ct + 1])

                # remaining weights, residual x (bf16), small vectors
                for w in (2, 0, 3):
                    for u in range(U):
                        nc.sync.dma_start(out=w_bf[w][u][:], in_=w_h[w][u])
                nc.sync.dma_start(out=bo4[:], in_=col4(bo_h))
                nc.sync.dma_start(out=bv4[:], in_=col4(bv_h))
                nc.sync.dma_start(out=bq8_0[:], in_=col8(bq_h))
                nc.sync.dma_start(out=bk8_0[:], in_=col8(bk_h))
                nc.sync.dma_start(out=bkc0[:], in_=col4(bk_h))
                for ct in range(CT):
                    nc.sync.dma_start(out=x_bf[ct][:], in_=x_t[ct])

                nc.vector.reciprocal(out=RA[:], in_=A4[:])
                nc.gpsimd.tensor_mul(DRA[:], D4[:], RA[:])
                dsrc = DRA[:]
                nc.gpsimd.tensor_copy(
                    dfa8[:],
                    bass.AP(tensor=dsrc.tensor, offset=dsrc.offset,
                            ap=[dsrc.ap[0], [1, 2], [2, U]]),
                )
                nc.gpsimd.tensor_copy(bv8c[:], bv4[:])

                # wq / wv scaling and wo conversion on Pool
                for cc in range(CT):
                    u, i = cc // 2, cc % 2
                    acol = A4[:, cc:cc + 1]
                    nc.gpsimd.tensor_scalar_mul(wf8[0][u][:, i, :],
                                                w_bf[0][u][:, i, :], acol)
                    nc.gpsimd.tensor_scalar_mul(wf8[2][u][:, i, :],
                                                w_bf[2][u][:, i, :], acol)
                for cc in range(CT):
                    nc.gpsimd.tensor_copy(wo8p[cc][:],
                                          w_bf[3][cc // 2][:, cc % 2, :])

                # bias folds b' = b + w @ d in [64,8] block-column layout
                def bias_fold8(w, b0, bout, name):
                    for ob in range(8):
                        pf = gnps.tile([64, 1], F32, tag="gn",
                                       name=f"bf{name}{ob}")
                        for u in range(U):
                            nc.tensor.matmul(
                                pf[:],
                                lhsT=wf8[w][u][:, :, ob * 64:(ob + 1) * 64],
                                rhs=dfa8[:, :, u:u + 1],
                                start=(u == 0), stop=(u == U - 1),
                                perf_mode=DR)
                        nc.vector.tensor_tensor(out=bout[:, ob:ob + 1],
                                                in0=pf[:],
                                                in1=b0[:, ob:ob + 1], op=ADD)

                bias_fold8(1, bk8_0, bk8, "k")

                # plain-k blocks need the folded bias in [128,4] col layout
                for b in range(CT):
                    pf4 = gnps.tile([P, 1], F32, tag="gn", name=f"bk4_{b}")
                    for cc in range(CT):
                        nc.tensor.matmul(
                            pf4[:],
                            lhsT=wf8[1][cc // 2][:, cc % 2, b * P:(b + 1) * P],
                            rhs=dfa8[:, cc % 2, cc // 2:cc // 2 + 1],
                            start=(cc == 0), stop=(cc == CT - 1))
                    nc.vector.tensor_tensor(out=bk4p[:, b:b + 1], in0=pf4[:],
                                            in1=bkc0[:, b:b + 1], op=ADD)

                bias_fold8(0, bq8_0, bq8, "q")

                # bo' = bo + wo @ bv (plain fp8)
                for ct_o in range(CT):
                    pbo = gnps.tile([P, 1], F32, tag="gn", name=f"bo{ct_o}")
                    for cc in range(CT):
                        nc.tensor.matmul(
                            pbo[:],
                            lhsT=wo8p[cc][:, ct_o * P:(ct_o + 1) * P],
                            rhs=bv8c[:, cc:cc + 1],
                            start=(cc == 0), stop=(cc == CT - 1))
                    nc.vector.tensor_tensor(out=bo4p[:, ct_o:ct_o + 1],
                                            in0=pbo[:],
                                            in1=bo4[:, ct_o:ct_o + 1], op=ADD)

                # ---- k / v projections, interleaved (k evac ACT, v DVE) ----
                def emit_k_jsl(jsl):
                    tsl = slice(jsl * 512, (jsl + 1) * 512)
                    if jsl >= 6:        # plain fp8: full-width psum
                        for b in range(CT):
                            psk = pjp.tile([P, 512], F32, tag="pj",
                                           name=f"k{jsl}_{b}")
                            for cc in range(CT):
                                nc.tensor.matmul(
                                    psk[:],
                                    lhsT=wf8[1][cc // 2][:, cc % 2,
                                                         b * P:(b + 1) * P],
                                    rhs=xf8[cc // 2][:, cc % 2, tsl],
                                    start=(cc == 0), stop=(cc == CT - 1))
                            nc.scalar.activation(
                                out=k2[b // 2][:, b % 2, tsl], in_=psk[:],
                                func=IDENT, scale=1.0,
                                bias=bk4p[:, b:b + 1])
                    else:               # DoubleRow
                        for ob in range(8):
                            psk = pjp.tile([P, 512], F32, tag="pj",
                                           name=f"k{jsl}_{ob}")
                            for u in range(U):
                                nc.tensor.matmul(
                                    psk[0:64, :],
                                    lhsT=wf8[1][u][:, :, ob * 64:(ob + 1) * 64],
                                    rhs=xf8[u][:, :, tsl],
                                    start=(u == 0), stop=(u == U - 1),
                                    perf_mode=DR)
                            po = (ob % 2) * 64
                            nc.scalar.activation(
                                out=k2[ob // 4][po:po + 64, (ob % 4) // 2, tsl],
                                in_=psk[0:64, :], func=IDENT,
                                scale=1.0, bias=bk8[:, ob:ob + 1])

                def emit_v_tb2(tb2):
                    m, i = tb2 // 2, tb2 % 2
                    if tb2 >= 22:       # plain fp8: full-width psum
                        psv = pjp.tile([P, 512], F32, tag="pj",
                                       name=f"vp{tb2}")
                        t0 = tb2 * P
                        for cc in range(CT):
                            nc.tensor.matmul(
                                psv[:],
                                lhsT=xf8[cc // 2][:, cc % 2, t0:t0 + P],
                                rhs=wf8[2][cc // 2][:, cc % 2, :],
                                start=(cc == 0), stop=(cc == CT - 1))
                        if tb2 % 2 == 0:
                            nc.scalar.mul(v2[m][:, i, :], psv[:], 1.0)
                        else:
                            nc.vector.tensor_copy(v2[m][:, i, :], psv[:])
                    else:               # DoubleRow, two 64-token halves
                        for hh in range(2):
                            psv = pjp.tile([P, 512], F32, tag="pj",
                                           name=f"v{tb2}_{hh}")
                            t0 = tb2 * P + hh * 64
                            for u in range(U):
                                nc.tensor.matmul(
                                    psv[0:64, :],
                                    lhsT=xf8[u][:, :, t0:t0 + 64],
                                    rhs=wf8[2][u][:],
                                    start=(u == 0), stop=(u == U - 1),
                                    perf_mode=DR)
                            po = hh * 64
                            nc.vector.tensor_copy(v2[m][po:po + 64, i, :],
                                                  psv[0:64, :])

                for g in range(8):
                    emit_k_jsl(g)
                    for j in range(4):
                        emit_v_tb2(4 * g + j)

                def emit_qproj(st, pool, tag, obs, on_act=False):
                    tsl = slice(st * 512, (st + 1) * 512)
                    for ob in obs:
                        psq = pool.tile([P, 512], F32, tag=tag,
                                        name=f"q{st}_{ob}")
                        for u in range(U):
                            nc.tensor.matmul(
                                psq[0:64, :],
                                lhsT=wf8[0][u][:, :, ob * 64:(ob + 1) * 64],
                                rhs=xf8[u][:, :, tsl],
                                start=(u == 0), stop=(u == U - 1),
                                perf_mode=DR)
                        po = (ob % 2) * 64
                        dst = q2[ob // 4][po:po + 64, (ob % 4) // 2, tsl]
                        if on_act:
                            nc.scalar.activation(out=dst, in_=psq[0:64, :],
                                                 func=IDENT, scale=1.0,
                                                 bias=bq8[:, ob:ob + 1])
                        else:
                            nc.vector.tensor_scalar_add(
                                out=dst, in0=psq[0:64, :],
                                scalar1=bq8[:, ob:ob + 1])

                emit_qproj(0, pjp, "pj", range(8), on_act=True)

            # =========== Phase C: attention strips ===========
            with tc.tile_pool(name="sc", bufs=7, space="PSUM") as scp, \
                 tc.tile_pool(name="lps", bufs=1, space="PSUM") as lp, \
                 tc.tile_pool(name="lsb", bufs=2) as lsp, \
                 tc.tile_pool(name="outt", bufs=3) as otp:

                rlb_of = {}
                psl_of = {}

                def emit_scores_unit(st, m, pT):
                    isl = slice(st * 512, (st + 1) * 512)
                    for jj in range(2):
                        jt = 2 * m + jj
                        if jt % 6 == 5:     # plain fp8 scores
                            ps = scp.tile([P, 512], F32, tag="sc",
                                          name=f"s{st}_{jt}")
                            j0 = jt * P
                            for cc in range(CT):
                                nc.tensor.matmul(
                                    ps[:],
                                    lhsT=k2[cc // 2][:, cc % 2, j0:j0 + P],
                                    rhs=q2[cc // 2][:, cc % 2, isl],
                                    start=(cc == 0), stop=(cc == CT - 1))
                            nc.scalar.activation(
                                out=pT[m][:, jj, :], in_=ps[:],
                                func=EXP, scale=SCALE, bias=mln16[:])
                        else:               # DoubleRow scores
                            for hh in range(2):
                                ps = scp.tile([P, 512], F32, tag="sc",
                                              name=f"s{st}_{jt}_{hh}")
                                j0 = jt * P + hh * 64
                                for u in range(U):
                                    nc.tensor.matmul(
                                        ps[0:64, :],
                                        lhsT=k2[u][:, :, j0:j0 + 64],
                                        rhs=q2[u][:, :, isl],
                                        start=(u == 0), stop=(u == U - 1),
                                        perf_mode=DR)
                                nc.scalar.activation(
                                    out=pT[m][hh * 64:(hh + 1) * 64, jj, :],
                                    in_=ps[0:64, :], func=EXP,
                                    scale=SCALE, bias=mln16[0:64, :])

                def emit_l_unit(st, m, pT):
                    # all-ones M=64 stationary: every psum row accumulates l,
                    # so the result is already broadcast across 64 partitions
                    nc.tensor.matmul(psl_of[st][0:64, :], lhsT=ones8[:],
                                     rhs=pT[m][:],
                                     start=(m == 0), stop=(m == M16 - 1),
                                     perf_mode=DR)

                def emit_rl_chain(st):
                    rlb = lsp.tile([64, 512], F32, tag="rlb", name=f"rlb{st}")
                    nc.vector.reciprocal(out=rlb[:], in_=psl_of[st][0:64, :])
                    rlb_of[st] = rlb

                def emit_h_chunk(sp, cb):
                    # h accumulation for channels cb*64.. of strip sp
                    i0 = sp * 512
                    pT = pT2[sp % 2]
                    psb_ = scp.tile([P, 512], F32, tag="sc",
                                    name=f"hB{sp}_{cb}")
                    for m in range(M16):
                        nc.tensor.matmul(
                            psb_[0:64, :],
                            lhsT=v2[m][:, :, cb * 64:(cb + 1) * 64],
                            rhs=pT[m][:],
                            start=(m == 0), stop=(m == M16 - 1),
                            perf_mode=DR)
                    po = (cb % 2) * 64
                    nc.vector.tensor_mul(
                        hT2[cb // 4][po:po + 64, (cb % 4) // 2, i0:i0 + 512],
                        psb_[0:64, :], rlb_of[sp][:])

                def emit_outproj(sp, blocks):
                    i0 = sp * 512
                    isl = slice(i0, i0 + 512)
                    for b in blocks:
                        pso = scp.tile([P, 512], F32, tag="sc",
                                       name=f"op{sp}_{b}")
                        for cc in range(CT):
                            nc.tensor.matmul(
                                pso[:],
                                lhsT=wo8p[cc][:, b * P:(b + 1) * P],
                                rhs=hT2[cc // 2][:, cc % 2, isl],
                                start=(cc == 0), stop=(cc == CT - 1))
                        ot = otp.tile([P, 512], F32, tag="ot",
                                      name=f"ot{sp}_{b}")
                        nc.vector.scalar_tensor_tensor(
                            out=ot[:], in0=pso[:], scalar=bo4p[:, b:b + 1],
                            in1=x_bf[b][:, isl], op0=ADD, op1=ADD)
                        nc.sync.dma_start(out=out_t[b][:, isl], in_=ot[:])

                hps3 = []

                for st in range(NSTRIP):
                    pT = pT2[st % 2]
                    psl_of[st] = lp.tile([P, 512], F32, tag="l",
                                         name=f"l{st}")
                    for m in range(M16):
                        emit_scores_unit(st, m, pT)
                        if m >= 2:
                            emit_l_unit(st, m - 2, pT)
                        if st > 0:
                            if m == 1:
                                emit_rl_chain(st - 1)
                            elif 2 <= m <= 9:
                                emit_h_chunk(st - 1, m - 2)
                            elif m in (10, 11):
                                emit_outproj(st - 1, (2 * (m - 10),
                                                      2 * (m - 10) + 1))
                        if st < NSTRIP - 1 and m in (12, 13):
                            emit_qproj(st + 1, scp, "sc",
                                       range(4 * (m - 12), 4 * (m - 11)))
                        if st == NSTRIP - 1 and m >= 10:
                            # start the first 4 h chunks of the last strip
                            # in-strip: 2 m-steps per unit per chunk
                            if m == 10:
                                hps3.extend(
                                    scp.tile([P, 512], F32, tag="sc",
                                             name=f"h3_{cb}")
                                    for cb in range(4))
                            for cb in range(4):
                                for mm_ in (2 * (m - 10), 2 * (m - 10) + 1):
                                    nc.tensor.matmul(
                                        hps3[cb][0:64, :],
                                        lhsT=v2[mm_][:, :,
                                                     cb * 64:(cb + 1) * 64],
                                        rhs=pT[mm_][:],
                                        start=(mm_ == 0), stop=False,
                                        perf_mode=DR)
                    emit_l_unit(st, M16 - 2, pT)
                    emit_l_unit(st, M16 - 1, pT)

                # tail: last strip's h chunks with the out-proj matmuls
                # interleaved (each cc half of hT2 becomes ready after two
                # chunks), then fused evacuation + store
                sp = NSTRIP - 1
                i0 = sp * 512
                isl = slice(i0, i0 + 512)
                emit_rl_chain(sp)
                # finish chunks 0-3 (m-steps 12..15), then evacuate
                for cb in range(4):
                    for mm_ in range(12, M16):
                        nc.tensor.matmul(
                            hps3[cb][0:64, :],
                            lhsT=v2[mm_][:, :, cb * 64:(cb + 1) * 64],
                            rhs=pT2[sp % 2][mm_][:],
                            start=False, stop=(mm_ == M16 - 1),
                            perf_mode=DR)
                for cb in range(4):
                    po = (cb % 2) * 64
                    nc.vector.tensor_mul(
                        hT2[0][po:po + 64, (cb % 4) // 2, isl],
                        hps3[cb][0:64, :], rlb_of[sp][:])
                op_ps = [scp.tile([P, 512], F32, tag="sc", name=f"opt_{b}")
                         for b in range(CT)]
                # out-proj first half (contracts hT2[0], ready now)
                for cc in (0, 1):
                    for b in range(CT):
                        nc.tensor.matmul(
                            op_ps[b][:],
                            lhsT=wo8p[cc][:, b * P:(b + 1) * P],
                            rhs=hT2[0][:, cc, isl],
                            start=(cc == 0), stop=False)
                for cb in range(4, 8):
                    emit_h_chunk(sp, cb)
                for cc in (2, 3):
                    for b in range(CT):
                        nc.tensor.matmul(
                            op_ps[b][:],
                            lhsT=wo8p[cc][:, b * P:(b + 1) * P],
                            rhs=hT2[1][:, cc - 2, isl],
                            start=False, stop=(cc == CT - 1))
                for b in range(CT):
                    ot = otp.tile([P, 512], F32, tag="ot", name=f"ott_{b}")
                    nc.vector.scalar_tensor_tensor(
                        out=ot[:], in0=op_ps[b][:], scalar=bo4p[:, b:b + 1],
                        in1=x_bf[b][:, isl], op0=ADD, op1=ADD)
                    nc.sync.dma_start(out=out_t[b][:, isl], in_=ot[:])

    nc.finalize()
    return nc


def kernel(**inputs):
    if "nc" not in _CACHE:
        _CACHE["nc"] = build_bass()
    nc = _CACHE["nc"]

    x = np.asarray(inputs["x"], dtype=np.float32)
    B = x.shape[0]
    xf = x.reshape(B, C, N)

    def to_dr(w):
        wT = np.asarray(w, dtype=np.float32).T        # [c, o]
        return wT.reshape(U, 2, P, C).transpose(0, 2, 1, 3)

    wdr = np.ascontiguousarray(
        np.stack([to_dr(inputs[k]) for k in ("wq", "wk", "wv", "wo")])
        .astype(ml_dtypes.bfloat16)
    )

    shared = {
        "wdr": wdr,
        "bq": np.ascontiguousarray(np.asarray(inputs["bq"], np.float32)),
        "bk": np.ascontiguousarray(np.asarray(inputs["bk"], np.float32)),
        "bv": np.ascontiguousarray(np.asarray(inputs["bv"], np.float32)),
        "bo": np.ascontiguousarray(np.asarray(inputs["bo"], np.float32)),
        "gam": np.ascontiguousarray(np.asarray(inputs["norm_g"], np.float32)),
        "bet": np.ascontiguousarray(np.asarray(inputs["norm_b"], np.float32)),
    }

    in_maps = []
    for core in range(2 * B):
        b, half = core // 2, core % 2
        xb = xf[b]
        if half:
            xb = np.concatenate([xb[:, NQ:], xb[:, :NQ]], axis=1)
        # fp8 copy in DoubleRow layout [u, p, i, n]
        xdr = np.ascontiguousarray(
            xb.reshape(U, 2, P, N).transpose(0, 2, 1, 3)
            .astype(ml_dtypes.float8_e4m3))
        in_maps.append(
            {"xbf": np.ascontiguousarray(xb.astype(ml_dtypes.bfloat16)),
             "xdr": xdr, **shared})

    import os
    trace = bool(os.environ.get("BASS_KERNEL_TRACE"))
    res = run_bass_kernel_spmd(
        nc, in_maps, core_ids=list(range(2 * B)), trace=trace,
        trace_cores=list(range(2 * B)) if trace else None,
    )
    _CACHE["last_results"] = res

    out = np.empty((B, C, N), np.float32)
    for core in range(2 * B):
        b, half = core // 2, core % 2
        out[b][:, half * NQ:(half + 1) * NQ] = res.results[core]["out"]
    return out.reshape(B, C, 64, 64)


# revision 66
# speedup vs baseline: 1.4969x; 1.0804x over previous
"""Trainium2 Bass kernel for nn_AttnBlock (GroupNorm + single-head 4096-token
attention + residual), sharded over 8 NeuronCores.

Sharding: data-parallel over batch B=4, sequence-parallel x2 over the 4096
query tokens -> 8 shards (token axis rolled on host for the second half so a
single SPMD NEFF serves all cores).

Compute strategy: fp8(e4m3) matmuls throughout. DoubleRow mode (0.5
cyc/row) is used where its 64-partition psum output is affordable;
plain-fp8 (1 cyc/row, full 128-partition psum) is mixed in to balance the
PE against the (ACT+DVE) evacuation/exp capacity:
  - scores: 27 of 32 key tiles per strip DoubleRow (2 exps of [64,512]
    with partition-offset pT writes), 5 plain (1 exp of [128,512]).
  - q/k/v projections: mostly DoubleRow, a slice plain; k-evacs on ACT,
    q/v-evacs on DVE, emission interleaved so both engines run.
  - p@v: DoubleRow; channels 0-255 accumulate in 4 psum banks during the
    strip (lagged 4 units behind scores so trailing v-evacs don't stall),
    channels 256-511 run as 4 16-matmul chunks through the scores pool
    during the NEXT strip (pT is double-buffered per strip, no WAR),
    each chunk evacuated immediately.
  - softmax denominator: fused ones-row DoubleRow matmul; 1/l applied at
    h-evacuation via a bf16 ones-matmul broadcast; the 1/l chain is
    emitted after the next strip's first unit so it never gates scores.
  - out-proj: plain fp8, evacuation fuses bias + residual in one DVE op.
GroupNorm's affine is folded into the q/k/v weights on device (w' = w*a,
bias' = b + w@d via tiny DoubleRow matmuls); per-chunk stats/fold math is
interleaved into the x DMA stream so the PE starts ~13us in. The v bias
folds into the out-proj bias (bo' = bo + wo@bv) so v evacuations are pure
copies; x is converted to fp8 once, overlapped with the x DMA (ACT/Pool).

Self-contained: hardcodes all shapes; only needs the concourse runtime.
"""

import numpy as np
import ml_dtypes

import concourse.bass as bass
import concourse.bacc as bacc
import concourse.tile as tile
from concourse import mybir
from concourse.bass_utils import run_bass_kernel_spmd

P = 128
C = 512
N = 4096
NQ = 2048
CT = 4                  # 128-channel chunks
U = 2                   # 256-channel DoubleRow pairs
JT = 32                 # key-token tiles of 128
M16 = 16                # key-token pair blocks of 256
NSTRIP = 4              # query strips of 512
HLAG = 4                # units of lag for the h accumulation
GS = 16
NG = P // GS
EPS = 1e-6
SCALE = float(C) ** -0.5
LN16 = 2.772588722239781
F32 = mybir.dt.float32
BF16 = mybir.dt.bfloat16
F8 = mybir.dt.float8e4
DR = mybir.MatmulPerfMode.DoubleRow
ADD = mybir.AluOpType.add
SUB = mybir.AluOpType.subtract
IDENT = mybir.ActivationFunctionType.Identity
EXP = mybir.ActivationFunctionType.Exp
SQRT = mybir.ActivationFunctionType.Sqrt

_CACHE = {}


def build_bass():
    nc = bacc.Bacc(None, target_bir_lowering=False)

    x_h = nc.dram_tensor("xbf", [C, N], BF16, kind="ExternalInput")[:]
    # xdr[u, p, i, n] = x[u*256 + i*128 + p, n] in fp8 (DoubleRow layout)
    xdr_h = nc.dram_tensor("xdr", [U, P, 2, N], F8, kind="ExternalInput")[:]
    # wdr[wi, u, p, i, o] = w_wi[o, u*256 + i*128 + p]; wi order q,k,v,o
    w_h = nc.dram_tensor("wdr", [4, U, P, 2, C], BF16, kind="ExternalInput")[:]
    bq_h = nc.dram_tensor("bq", [C], F32, kind="ExternalInput")[:]
    bk_h = nc.dram_tensor("bk", [C], F32, kind="ExternalInput")[:]
    bv_h = nc.dram_tensor("bv", [C], F32, kind="ExternalInput")[:]
    bo_h = nc.dram_tensor("bo", [C], F32, kind="ExternalInput")[:]
    gam_h = nc.dram_tensor("gam", [C], F32, kind="ExternalInput")[:]
    bet_h = nc.dram_tensor("bet", [C], F32, kind="ExternalInput")[:]
    out_h = nc.dram_tensor("out", [C, NQ], F32, kind="ExternalOutput")[:]

    g8_np = np.zeros((P, NG), np.float32)
    g8T_np = np.zeros((NG, P), np.float32)
    for c in range(P):
        g8_np[c, c // GS] = 1.0 / GS
        g8T_np[c // GS, c] = 1.0
    g8_h = nc.inline_tensor(g8_np, name="g8")[:]
    g8T_h = nc.inline_tensor(g8T_np, name="g8T")[:]

    x_t = x_h.rearrange("(t p) n -> t p n", p=P)
    out_t = out_h.rearrange("(t p) n -> t p n", p=P)

    def col4(ap1d):
        return bass.AP(tensor=ap1d.tensor, offset=ap1d.offset, ap=[[1, P], [P, CT]])

    def col8(ap1d):
        return bass.AP(tensor=ap1d.tensor, offset=ap1d.offset, ap=[[1, 64], [64, 8]])

    with tile.TileContext(nc) as tc:
        with tc.tile_pool(name="consts", bufs=1) as cp, \
             tc.tile_pool(name="xbf", bufs=1) as xbp, \
             tc.tile_pool(name="xf8", bufs=1) as x8p, \
             tc.tile_pool(name="wf8", bufs=1) as w8p, \
             tc.tile_pool(name="qkv", bufs=1) as qkvp, \
             tc.tile_pool(name="hT", bufs=1) as hTp, \
             tc.tile_pool(name="pT", bufs=1) as pTp:

            # ---- constants ----
            ones8 = cp.tile([P, 2, 64], F8, tag="ones8")
            nc.vector.memset(ones8[:], 1.0)
            eps_t = cp.tile([P, 1], F32, tag="eps")
            nc.vector.memset(eps_t[:], EPS)
            mln16 = cp.tile([P, 1], F32, tag="mln16")
            nc.vector.memset(mln16[:], -LN16)
            g8_sb = cp.tile([P, NG], F32, tag="g8")
            nc.sync.dma_start(out=g8_sb[:], in_=g8_h)
            g8T_sb = cp.tile([NG, P], F32, tag="g8T")
            nc.sync.dma_start(out=g8T_sb[:], in_=g8T_h)
            gam_sb = cp.tile([P, CT], F32, tag="gam")
            bet_sb = cp.tile([P, CT], F32, tag="bet")
            bo4 = cp.tile([P, CT], F32, tag="bo4")
            bo4p = cp.tile([P, CT], F32, tag="bo4p")
            bv4 = cp.tile([P, CT], F32, tag="bv4")
            bv8c = cp.tile([P, CT], F8, tag="bv8c")
            bq8_0 = cp.tile([64, 8], F32, tag="bq8_0")
            bk8_0 = cp.tile([64, 8], F32, tag="bk8_0")
            bq8 = cp.tile([64, 8], F32, tag="bq8")
            bk8 = cp.tile([64, 8], F32, tag="bk8")
            bkc0 = cp.tile([P, CT], F32, tag="bkc0")
            bk4p = cp.tile([P, CT], F32, tag="bk4p")
            A4 = cp.tile([P, CT], F32, tag="A4")
            D4 = cp.tile([P, CT], F32, tag="D4")
            RA = cp.tile([P, CT], F32, tag="RA")
            DRA = cp.tile([P, CT], F32, tag="DRA")
            dfa8 = cp.tile([P, 2, U], F8, tag="dfa8")

            # ---- persistent activations ----
            x_bf = [xbp.tile([P, N], BF16, tag=f"x{t}", name=f"x{t}")
                    for t in range(CT)]
            xf8 = [x8p.tile([P, 2, N], F8, tag=f"x8{u}", name=f"x8{u}")
                   for u in range(U)]
            wf8 = [[w8p.tile([P, 2, C], F8, tag=f"w8_{w}{u}", name=f"w8_{w}{u}")
                    for u in range(U)] for w in range(3)]
            wo8p = [w8p.tile([P, C], F8, tag=f"wo8_{t}", name=f"wo8_{t}")
                    for t in range(CT)]
            q2 = [qkvp.tile([P, 2, NQ], F8, tag=f"q{u}", name=f"q{u}")
                  for u in range(U)]
            k2 = [qkvp.tile([P, 2, N], F8, tag=f"k{u}", name=f"k{u}")
                  for u in range(U)]
            v2 = [qkvp.tile([P, 2, C], F8, tag=f"v{m}", name=f"v{m}")
                  for m in range(M16)]
            hT2 = [hTp.tile([P, 2, NQ], F8, tag=f"hT{u}", name=f"hT{u}")
                   for u in range(U)]
            pT2 = [[pTp.tile([P, 2, 512], F8, tag=f"pT{b}_{m}",
                             name=f"pT{b}_{m}") for m in range(M16)]
                   for b in range(2)]

            # =========== Phase A/B ===========
            with tc.tile_pool(name="wbf", bufs=1) as wbp, \
                 tc.tile_pool(name="gn", bufs=2) as gnp, \
                 tc.tile_pool(name="gnps", bufs=2, space="PSUM") as gnps, \
                 tc.tile_pool(name="pj", bufs=6, space="PSUM") as pjp:

                w_bf = [[wbp.tile([P, 2, C], BF16, tag=f"wb{w}{u}",
                                  name=f"wb{w}{u}")
                         for u in range(U)] for w in range(4)]

                # x (fp8, host-formatted) first, then wk + gn affine vectors
                for ct in range(CT):
                    u, i = ct // 2, ct % 2
                    nc.sync.dma_start(out=xf8[u][:, i, :], in_=xdr_h[u][:, i, :])
                for u in range(U):
                    nc.sync.dma_start(out=w_bf[1][u][:], in_=w_h[1][u])
                nc.sync.dma_start(out=gam_sb[:], in_=col4(gam_h))
                nc.sync.dma_start(out=bet_sb[:], in_=col4(bet_h))

                stats = gnp.tile([P, CT, 8, 6], F32, tag="stats")
                asums = gnp.tile([P, 2], F32, tag="asums")
                ascr = gnp.tile([P, N], F8, tag="ascr")

                def gn_math(ct):
                    cstat = gnp.tile([P, 2], F32, tag="cstat", name=f"cs{ct}")
                    if ct == CT - 1:
                        # ct3 stats arrive as [sum(x), sum(x^2)] from ACT
                        nc.vector.tensor_scalar_mul(cstat[:], asums[:],
                                                    1.0 / N)
                    else:
                        mv = gnp.tile([P, 2], F32, tag="mv", name=f"mv{ct}")
                        nc.vector.bn_aggr(out=mv[:], in_=stats[:, ct, :, :])
                        nc.vector.tensor_copy(cstat[:, 0:1], mv[:, 0:1])
                        nc.vector.tensor_mul(cstat[:, 1:2], mv[:, 0:1],
                                             mv[:, 0:1])
                        nc.vector.tensor_add(cstat[:, 1:2], cstat[:, 1:2],
                                             mv[:, 1:2])
                    psA = gnps.tile([NG, 2], F32, tag="gn", name=f"gA{ct}")
                    nc.tensor.matmul(psA[:], lhsT=g8_sb[:], rhs=cstat[:],
                                     start=True, stop=True)
                    gt = gnp.tile([NG, 2], F32, tag="gt", name=f"gt{ct}")
                    nc.vector.tensor_copy(gt[:], psA[:])
                    psB = gnps.tile([P, 2], F32, tag="gn", name=f"gB{ct}")
                    nc.tensor.matmul(psB[:], lhsT=g8T_sb[:], rhs=gt[:],
                                     start=True, stop=True)
                    gstat = gnp.tile([P, 2], F32, tag="gstat", name=f"gs{ct}")
                    nc.vector.tensor_copy(gstat[:], psB[:])
                    # var + eps, then rsqrt via reciprocal seed + 2 Newton
                    # steps (avoids the Sqrt activation: keeping every ACT
                    # func in the exp table set avoids a mid-kernel
                    # LoadActFuncSet switch)
                    vtmp = gnp.tile([P, 1], F32, tag="vtmp", name=f"vt{ct}")
                    nc.vector.tensor_mul(vtmp[:], gstat[:, 0:1], gstat[:, 0:1])
                    nc.vector.tensor_tensor(out=vtmp[:], in0=gstat[:, 1:2],
                                            in1=vtmp[:], op=SUB)
                    nc.vector.tensor_scalar_add(out=vtmp[:], in0=vtmp[:],
                                                scalar1=EPS)
                    rstd = gnp.tile([P, 1], F32, tag="rstd", name=f"rs{ct}")
                    nc.vector.reciprocal(out=rstd[:], in_=vtmp[:])
                    nt = gnp.tile([P, 1], F32, tag="nt", name=f"nt{ct}")
                    for _ in range(2):
                        nc.vector.tensor_mul(nt[:], rstd[:], rstd[:])
                        nc.vector.tensor_mul(nt[:], nt[:], vtmp[:])
                        nc.vector.tensor_scalar(out=nt[:], in0=nt[:],
                                                scalar1=-0.5, scalar2=1.5,
                                                op0=mybir.AluOpType.mult,
                                                op1=ADD)
                        nc.vector.tensor_mul(rstd[:], rstd[:], nt[:])
                    nc.vector.tensor_mul(A4[:, ct:ct + 1], rstd[:],
                                         gam_sb[:, ct:ct + 1])
                    dt_ = gnp.tile([P, 1], F32, tag="dt", name=f"dt{ct}")
                    nc.vector.tensor_mul(dt_[:], gstat[:, 0:1], A4[:, ct:ct + 1])
                    nc.vector.tensor_tensor(out=D4[:, ct:ct + 1],
                                            in0=bet_sb[:, ct:ct + 1],
                                            in1=dt_[:], op=SUB)

                # stats straight off the fp8 x; gn math + wk-scale per chunk.
                # ct3's sums run on the otherwise-idle ACT engine so the
                # DVE-serial stats chain is ~25% shorter.
                nc.scalar.activation(out=ascr[:], in_=xf8[1][:, 1, :],
                                     func=mybir.ActivationFunctionType.Copy,
                                     accum_out=asums[:, 0:1])
                nc.scalar.activation(out=ascr[:], in_=xf8[1][:, 1, :],
                                     func=mybir.ActivationFunctionType.Square,
                                     accum_out=asums[:, 1:2])
                for ct in range(CT):
                    u, i = ct // 2, ct % 2
                    if ct < CT - 1:
                        for s8 in range(8):
                            nc.vector.bn_stats(
                                out=stats[:, ct, s8, :],
                                in_=xf8[u][:, i, s8 * 512:(s8 + 1) * 512],
                            )
                    gn_math(ct)
                    nc.vector.tensor_scalar_mul(wf8[1][u][:, i, :],
                                                w_bf[1][u][:, i, :],
                                                A4[:, ct:ct + 1])

                # remaining weights, residual x (bf16), small vectors
                for w in (2, 0, 3):
                    for u in range(U):
                        nc.sync.dma_start(out=w_bf[w][u][:], in_=w_h[w][u])
                nc.sync.dma_start(out=bo4[:], in_=col4(bo_h))
                nc.sync.dma_start(out=bv4[:], in_=col4(bv_h))
                nc.sync.dma_start(out=bq8_0[:], in_=col8(bq_h))
                nc.sync.dma_start(out=bk8_0[:], in_=col8(bk_h))
                nc.sync.dma_start(out=bkc0[:], in_=col4(bk_h))
                for ct in range(CT):
                    nc.sync.dma_start(out=x_bf[ct][:], in_=x_t[ct])

                nc.vector.reciprocal(out=RA[:], in_=A4[:])
                nc.gpsimd.tensor_mul(DRA[:], D4[:], RA[:])
                dsrc = DRA[:]
                nc.gpsimd.tensor_copy(
                    dfa8[:],
                    bass.AP(tensor=dsrc.tensor, offset=dsrc.offset,
                            ap=[dsrc.ap[0], [1, 2], [2, U]]),
                )
                nc.gpsimd.tensor_copy(bv8c[:], bv4[:])

                # wq / wv scaling and wo conversion on Pool
                for cc in range(CT):
                    u, i = cc // 2, cc % 2
                    acol = A4[:, cc:cc + 1]
                    nc.gpsimd.tensor_scalar_mul(wf8[0][u][:, i, :],
                                                w_bf[0][u][:, i, :], acol)
                    nc.gpsimd.tensor_scalar_mul(wf8[2][u][:, i, :],
                                                w_bf[2][u][:, i, :], acol)
                for cc in range(CT):
                    nc.gpsimd.tensor_copy(wo8p[cc][:],
                                          w_bf[3][cc // 2][:, cc % 2, :])

                # bias folds b' = b + w @ d in [64,8] block-column layout
                def bias_fold8(w, b0, bout, name):
                    for ob in range(8):
                        pf = gnps.tile([64, 1], F32, tag="gn",
                                       name=f"bf{name}{ob}")
                        for u in range(U):
                            nc.tensor.matmul(
                                pf[:],
                                lhsT=wf8[w][u][:, :, ob * 64:(ob + 1) * 64],
                                rhs=dfa8[:, :, u:u + 1],
                                start=(u == 0), stop=(u == U - 1),
                                perf_mode=DR)
                        nc.scalar.activation(out=bout[:, ob:ob + 1],
                                             in_=pf[:], func=IDENT,
                                             scale=1.0,
                                             bias=b0[:, ob:ob + 1])

                bias_fold8(1, bk8_0, bk8, "k")

                # plain-k blocks need the folded bias in [128,4] col layout
                for b in range(CT):
                    pf4 = gnps.tile([P, 1], F32, tag="gn", name=f"bk4_{b}")
                    for cc in range(CT):
                        nc.tensor.matmul(
                            pf4[:],
                            lhsT=wf8[1][cc // 2][:, cc % 2, b * P:(b + 1) * P],
                            rhs=dfa8[:, cc % 2, cc // 2:cc // 2 + 1],
                            start=(cc == 0), stop=(cc == CT - 1))
                    nc.scalar.activation(out=bk4p[:, b:b + 1], in_=pf4[:],
                                         func=IDENT, scale=1.0,
                                         bias=bkc0[:, b:b + 1])

                bias_fold8(0, bq8_0, bq8, "q")

                # bo' = bo + wo @ bv (plain fp8)
                for ct_o in range(CT):
                    pbo = gnps.tile([P, 1], F32, tag="gn", name=f"bo{ct_o}")
                    for cc in range(CT):
                        nc.tensor.matmul(
                            pbo[:],
                            lhsT=wo8p[cc][:, ct_o * P:(ct_o + 1) * P],
                            rhs=bv8c[:, cc:cc + 1],
                            start=(cc == 0), stop=(cc == CT - 1))
                    nc.scalar.activation(out=bo4p[:, ct_o:ct_o + 1],
                                         in_=pbo[:], func=IDENT, scale=1.0,
                                         bias=bo4[:, ct_o:ct_o + 1])

                # ---- k / v projections, interleaved (k evac ACT, v DVE) ----
                def emit_k_jsl(jsl):
                    tsl = slice(jsl * 512, (jsl + 1) * 512)
                    if jsl >= 6:        # plain fp8: full-width psum
                        for b in range(CT):
                            psk = pjp.tile([P, 512], F32, tag="pj",
                                           name=f"k{jsl}_{b}")
                            for cc in range(CT):
                                nc.tensor.matmul(
                                    psk[:],
                                    lhsT=wf8[1][cc // 2][:, cc % 2,
                                                         b * P:(b + 1) * P],
                                    rhs=xf8[cc // 2][:, cc % 2, tsl],
                                    start=(cc == 0), stop=(cc == CT - 1))
                            if b % 2 == 0:
                                nc.scalar.activation(
                                    out=k2[b // 2][:, b % 2, tsl], in_=psk[:],
                                    func=IDENT, scale=1.0,
                                    bias=bk4p[:, b:b + 1])
                            else:
                                nc.vector.tensor_scalar_add(
                                    out=k2[b // 2][:, b % 2, tsl],
                                    in0=psk[:], scalar1=bk4p[:, b:b + 1])
                    else:               # DoubleRow, evacs split ACT/DVE
                        for ob in range(8):
                            psk = pjp.tile([P, 512], F32, tag="pj",
                                           name=f"k{jsl}_{ob}")
                            for u in range(U):
                                nc.tensor.matmul(
                                    psk[0:64, :],
                                    lhsT=wf8[1][u][:, :, ob * 64:(ob + 1) * 64],
                                    rhs=xf8[u][:, :, tsl],
                                    start=(u == 0), stop=(u == U - 1),
                                    perf_mode=DR)
                            po = (ob % 2) * 64
                            dst = k2[ob // 4][po:po + 64, (ob % 4) // 2, tsl]
                            if ob % 2 == 0:
                                nc.scalar.activation(
                                    out=dst, in_=psk[0:64, :], func=IDENT,
                                    scale=1.0, bias=bk8[:, ob:ob + 1])
                            else:
                                nc.vector.tensor_scalar_add(
                                    out=dst, in0=psk[0:64, :],
                                    scalar1=bk8[:, ob:ob + 1])

                def emit_v_tb2(tb2, pool, tag):
                    m, i = tb2 // 2, tb2 % 2
                    if tb2 >= 18:       # plain fp8: full-width psum
                        psv = pool.tile([P, 512], F32, tag=tag,
                                        name=f"vp{tb2}")
                        t0 = tb2 * P
                        for cc in range(CT):
                            nc.tensor.matmul(
                                psv[:],
                                lhsT=xf8[cc // 2][:, cc % 2, t0:t0 + P],
                                rhs=wf8[2][cc // 2][:, cc % 2, :],
                                start=(cc == 0), stop=(cc == CT - 1))
                        nc.vector.tensor_copy(v2[m][:, i, :], psv[:])
                    else:               # DoubleRow, two 64-token halves
                        for hh in range(2):
                            psv = pool.tile([P, 512], F32, tag=tag,
                                            name=f"v{tb2}_{hh}")
                            t0 = tb2 * P + hh * 64
                            for u in range(U):
                                nc.tensor.matmul(
                                    psv[0:64, :],
                                    lhsT=xf8[u][:, :, t0:t0 + 64],
                                    rhs=wf8[2][u][:],
                                    start=(u == 0), stop=(u == U - 1),
                                    perf_mode=DR)
                            po = hh * 64
                            nc.vector.tensor_copy(v2[m][po:po + 64, i, :],
                                                  psv[0:64, :])

                for g in range(8):
                    emit_k_jsl(g)

                def emit_qproj(st, pool, tag, obs, on_act=False):
                    tsl = slice(st * 512, (st + 1) * 512)
                    for ob in obs:
                        psq = pool.tile([P, 512], F32, tag=tag,
                                        name=f"q{st}_{ob}")
                        for u in range(U):
                            nc.tensor.matmul(
                                psq[0:64, :],
                                lhsT=wf8[0][u][:, :, ob * 64:(ob + 1) * 64],
                                rhs=xf8[u][:, :, tsl],
                                start=(u == 0), stop=(u == U - 1),
                                perf_mode=DR)
                        po = (ob % 2) * 64
                        dst = q2[ob // 4][po:po + 64, (ob % 4) // 2, tsl]
                        if on_act:
                            nc.scalar.activation(out=dst, in_=psq[0:64, :],
                                                 func=IDENT, scale=1.0,
                                                 bias=bq8[:, ob:ob + 1])
                        else:
                            nc.vector.tensor_scalar_add(
                                out=dst, in0=psq[0:64, :],
                                scalar1=bq8[:, ob:ob + 1])

                emit_qproj(0, pjp, "pj", range(8), on_act=True)

            # =========== Phase C: attention strips ===========
            with tc.tile_pool(name="sc", bufs=7, space="PSUM") as scp, \
                 tc.tile_pool(name="lps", bufs=1, space="PSUM") as lp, \
                 tc.tile_pool(name="lsb", bufs=2) as lsp, \
                 tc.tile_pool(name="outt", bufs=3) as otp:

                rlb_of = {}
                psl_of = {}

                def emit_scores_unit(st, m, pT):
                    isl = slice(st * 512, (st + 1) * 512)
                    for jj in range(2):
                        jt = 2 * m + jj
                        if jt % 6 == 5:     # plain fp8 scores
                            ps = scp.tile([P, 512], F32, tag="sc",
                                          name=f"s{st}_{jt}")
                            j0 = jt * P
                            for cc in range(CT):
                                nc.tensor.matmul(
                                    ps[:],
                                    lhsT=k2[cc // 2][:, cc % 2, j0:j0 + P],
                                    rhs=q2[cc // 2][:, cc % 2, isl],
                                    start=(cc == 0), stop=(cc == CT - 1))
                            nc.scalar.activation(
                                out=pT[m][:, jj, :], in_=ps[:],
                                func=EXP, scale=SCALE, bias=mln16[:])
                        else:               # DoubleRow scores
                            for hh in range(2):
                                ps = scp.tile([P, 512], F32, tag="sc",
                                              name=f"s{st}_{jt}_{hh}")
                                j0 = jt * P + hh * 64
                                for u in range(U):
                                    nc.tensor.matmul(
                                        ps[0:64, :],
                                        lhsT=k2[u][:, :, j0:j0 + 64],
                                        rhs=q2[u][:, :, isl],
                                        start=(u == 0), stop=(u == U - 1),
                                        perf_mode=DR)
                                nc.scalar.activation(
                                    out=pT[m][hh * 64:(hh + 1) * 64, jj, :],
                                    in_=ps[0:64, :], func=EXP,
                                    scale=SCALE, bias=mln16[0:64, :])

                def emit_l_unit(st, m, pT):
                    # all-ones M=64 stationary: every psum row accumulates l,
                    # so the result is already broadcast across 64 partitions
                    nc.tensor.matmul(psl_of[st][0:64, :], lhsT=ones8[:],
                                     rhs=pT[m][:],
                                     start=(m == 0), stop=(m == M16 - 1),
                                     perf_mode=DR)

                def emit_rl_chain(st):
                    rlb = lsp.tile([64, 512], F32, tag="rlb", name=f"rlb{st}")
                    nc.vector.reciprocal(out=rlb[:], in_=psl_of[st][0:64, :])
                    rlb_of[st] = rlb

                def emit_h_chunk(sp, cb):
                    # h accumulation for channels cb*64.. of strip sp
                    i0 = sp * 512
                    pT = pT2[sp % 2]
                    psb_ = scp.tile([P, 512], F32, tag="sc",
                                    name=f"hB{sp}_{cb}")
                    for m in range(M16):
                        nc.tensor.matmul(
                            psb_[0:64, :],
                            lhsT=v2[m][:, :, cb * 64:(cb + 1) * 64],
                            rhs=pT[m][:],
                            start=(m == 0), stop=(m == M16 - 1),
                            perf_mode=DR)
                    po = (cb % 2) * 64
                    nc.vector.tensor_mul(
                        hT2[cb // 4][po:po + 64, (cb % 4) // 2, i0:i0 + 512],
                        psb_[0:64, :], rlb_of[sp][:])

                def emit_outproj(sp, blocks):
                    i0 = sp * 512
                    isl = slice(i0, i0 + 512)
                    for b in blocks:
                        pso = scp.tile([P, 512], F32, tag="sc",
                                       name=f"op{sp}_{b}")
                        for cc in range(CT):
                            nc.tensor.matmul(
                                pso[:],
                                lhsT=wo8p[cc][:, b * P:(b + 1) * P],
                                rhs=hT2[cc // 2][:, cc % 2, isl],
                                start=(cc == 0), stop=(cc == CT - 1))
                        ot = otp.tile([P, 512], F32, tag="ot",
                                      name=f"ot{sp}_{b}")
                        nc.vector.scalar_tensor_tensor(
                            out=ot[:], in0=pso[:], scalar=bo4p[:, b:b + 1],
                            in1=x_bf[b][:, isl], op0=ADD, op1=ADD)
                        nc.sync.dma_start(out=out_t[b][:, isl], in_=ot[:])

                hps3 = []

                for st in range(NSTRIP):
                    pT = pT2[st % 2]
                    psl_of[st] = lp.tile([P, 512], F32, tag="l",
                                         name=f"l{st}")
                    for m in range(M16):
                        emit_scores_unit(st, m, pT)
                        if m >= 2:
                            emit_l_unit(st, m - 2, pT)
                        if st > 0:
                            if m == 1:
                                emit_rl_chain(st - 1)
                            elif 2 <= m <= 9:
                                emit_h_chunk(st - 1, m - 2)
                            elif m in (10, 11):
                                emit_outproj(st - 1, (2 * (m - 10),
                                                      2 * (m - 10) + 1))
                        if st == 0:
                            emit_v_tb2(2 * m, scp, "sc")
                            emit_v_tb2(2 * m + 1, scp, "sc")
                        if st < NSTRIP - 1 and m in (12, 13):
                            emit_qproj(st + 1, scp, "sc",
                                       range(4 * (m - 12), 4 * (m - 11)))
                        if st == NSTRIP - 1 and m >= 10:
                            # start the first 4 h chunks of the last strip
                            # in-strip: 2 m-steps per unit per chunk
                            if m == 10:
                                hps3.extend(
                                    scp.tile([P, 512], F32, tag="sc",
                                             name=f"h3_{cb}")
                                    for cb in range(4))
                            for cb in range(4):
                                for mm_ in (2 * (m - 10), 2 * (m - 10) + 1):
                                    nc.tensor.matmul(
                                        hps3[cb][0:64, :],
                                        lhsT=v2[mm_][:, :,
                                                     cb * 64:(cb + 1) * 64],
                                        rhs=pT[mm_][:],
                                        start=(mm_ == 0), stop=False,
                                        perf_mode=DR)
                    emit_l_unit(st, M16 - 2, pT)
                    emit_l_unit(st, M16 - 1, pT)

                # tail: last strip's h chunks with the out-proj matmuls
                # interleaved (each cc half of hT2 becomes ready after two
                # chunks), then fused evacuation + store
                sp = NSTRIP - 1
                i0 = sp * 512
                isl = slice(i0, i0 + 512)
                emit_rl_chain(sp)
                # finish chunks 0-3 (m-steps 12..15), then evacuate
                for cb in range(4):
                    for mm_ in range(12, M16):
                        nc.tensor.matmul(
                            hps3[cb][0:64, :],
                            lhsT=v2[mm_][:, :, cb * 64:(cb + 1) * 64],
                            rhs=pT2[sp % 2][mm_][:],
                            start=False, stop=(mm_ == M16 - 1),
                            perf_mode=DR)
                for cb in range(4):
                    po = (cb % 2) * 64
                    nc.vector.tensor_mul(
                        hT2[0][po:po + 64, (cb % 4) // 2, isl],
                        hps3[cb][0:64, :], rlb_of[sp][:])
                op_ps = [scp.tile([P, 512], F32, tag="sc", name=f"opt_{b}")
                         for b in range(CT)]
                # out-proj first half (contracts hT2[0], ready now)
                for cc in (0, 1):
                    for b in range(CT):
                        nc.tensor.matmul(
                            op_ps[b][:],
                            lhsT=wo8p[cc][:, b * P:(b + 1) * P],
                            rhs=hT2[0][:, cc, isl],
                            start=(cc == 0), stop=False)
                for cb in range(4, 8):
                    emit_h_chunk(sp, cb)
                for cc in (2, 3):
                    for b in range(CT):
                        nc.tensor.matmul(
                            op_ps[b][:],
                            lhsT=wo8p[cc][:, b * P:(b + 1) * P],
                            rhs=hT2[1][:, cc - 2, isl],
                            start=False, stop=(cc == CT - 1))
                for b in range(CT):
                    ot = otp.tile([P, 512], F32, tag="ot", name=f"ott_{b}")
                    for hf in range(2):
                        c0, c1 = i0 + hf * 256, i0 + (hf + 1) * 256
                        nc.vector.scalar_tensor_tensor(
                            out=ot[:, hf * 256:(hf + 1) * 256],
                            in0=op_ps[b][:, hf * 256:(hf + 1) * 256],
                            scalar=bo4p[:, b:b + 1],
                            in1=x_bf[b][:, c0:c1], op0=ADD, op1=ADD)
                        nc.sync.dma_start(out=out_t[b][:, c0:c1],
                                          in_=ot[:, hf * 256:(hf + 1) * 256])

    nc.finalize()
    return nc


def kernel(**inputs):
    if "nc" not in _CACHE:
        _CACHE["nc"] = build_bass()
    nc = _CACHE["nc"]

    x = np.asarray(inputs["x"], dtype=np.float32)
    B = x.shape[0]
    xf = x.reshape(B, C, N)

    def to_dr(w):
        wT = np.asarray(w, dtype=np.float32).T        # [c, o]
        return wT.reshape(U, 2, P, C).transpose(0, 2, 1, 3)

    wdr = np.ascontiguousarray(
        np.stack([to_dr(inputs[k]) for k in ("wq", "wk", "wv", "wo")])
        .astype(ml_dtypes.bfloat16)
    )

    shared = {
        "wdr": wdr,
        "bq": np.ascontiguousarray(np.asarray(inputs["bq"], np.float32)),
        "bk": np.ascontiguousarray(np.asarray(inputs["bk"], np.float32)),
        "bv": np.ascontiguousarray(np.asarray(inputs["bv"], np.float32)),
        "bo": np.ascontiguousarray(np.asarray(inputs["bo"], np.float32)),
        "gam": np.ascontiguousarray(np.asarray(inputs["norm_g"], np.float32)),
        "bet": np.ascontiguousarray(np.asarray(inputs["norm_b"], np.float32)),
    }

    in_maps = []
    for core in range(2 * B):
        b, half = core // 2, core % 2
        xb = xf[b]
        if half:
            xb = np.concatenate([xb[:, NQ:], xb[:, :NQ]], axis=1)
        # fp8 copy in DoubleRow layout [u, p, i, n]
        xdr = np.ascontiguousarray(
            xb.reshape(U, 2, P, N).transpose(0, 2, 1, 3)
            .astype(ml_dtypes.float8_e4m3))
        in_maps.append(
            {"xbf": np.ascontiguousarray(xb.astype(ml_dtypes.bfloat16)),
             "xdr": xdr, **shared})

    import os
    trace = bool(os.environ.get("BASS_KERNEL_TRACE"))
    res = run_bass_kernel_spmd(
        nc, in_maps, core_ids=list(range(2 * B)), trace=trace,
        trace_cores=list(range(2 * B)) if trace else None,
    )
    _CACHE["last_results"] = res

    out = np.empty((B, C, N), np.float32)
    for core in range(2 * B):
        b, half = core // 2, core % 2
        out[b][:, half * NQ:(half + 1) * NQ] = res.results[core]["out"]
    return out.reshape(B, C, 64, 64)


# revision 76
# speedup vs baseline: 1.5257x; 1.0192x over previous
"""Trainium2 Bass kernel for nn_AttnBlock (GroupNorm + single-head 4096-token
attention + residual), sharded over 8 NeuronCores.

Sharding: data-parallel over batch B=4, sequence-parallel x2 over the 4096
query tokens -> 8 shards (token axis rolled on host for the second half so a
single SPMD NEFF serves all cores).

Compute strategy: fp8(e4m3) matmuls throughout. DoubleRow mode (0.5
cyc/row) is used where its 64-partition psum output is affordable;
plain-fp8 (1 cyc/row, full 128-partition psum) is mixed in to balance the
PE against the (ACT+DVE) evacuation/exp capacity:
  - scores: 27 of 32 key tiles per strip DoubleRow (2 exps of [64,512]
    with partition-offset pT writes), 5 plain (1 exp of [128,512]).
  - q/k/v projections: mostly DoubleRow, a slice plain; k-evacs on ACT,
    q/v-evacs on DVE, emission interleaved so both engines run.
  - p@v: DoubleRow; channels 0-255 accumulate in 4 psum banks during the
    strip (lagged 4 units behind scores so trailing v-evacs don't stall),
    channels 256-511 run as 4 16-matmul chunks through the scores pool
    during the NEXT strip (pT is double-buffered per strip, no WAR),
    each chunk evacuated immediately.
  - softmax denominator: fused ones-row DoubleRow matmul; 1/l applied at
    h-evacuation via a bf16 ones-matmul broadcast; the 1/l chain is
    emitted after the next strip's first unit so it never gates scores.
  - out-proj: plain fp8, evacuation fuses bias + residual in one DVE op.
GroupNorm's affine is folded into the q/k/v weights on device (w' = w*a,
bias' = b + w@d via tiny DoubleRow matmuls); per-chunk stats/fold math is
interleaved into the x DMA stream so the PE starts ~13us in. The v bias
folds into the out-proj bias (bo' = bo + wo@bv) so v evacuations are pure
copies; x is converted to fp8 once, overlapped with the x DMA (ACT/Pool).

Self-contained: hardcodes all shapes; only needs the concourse runtime.
"""

import numpy as np
import ml_dtypes

import concourse.bass as bass
import concourse.bacc as bacc
import concourse.tile as tile
from concourse import mybir
from concourse.bass_utils import run_bass_kernel_spmd

P = 128
C = 512
N = 4096
NQ = 2048
CT = 4                  # 128-channel chunks
U = 2                   # 256-channel DoubleRow pairs
JT = 32                 # key-token tiles of 128
M16 = 16                # key-token pair blocks of 256
NSTRIP = 4              # query strips of 512
HLAG = 4                # units of lag for the h accumulation
GS = 16
NG = P // GS
EPS = 1e-6
SCALE = float(C) ** -0.5
LN16 = 2.772588722239781
F32 = mybir.dt.float32
BF16 = mybir.dt.bfloat16
F8 = mybir.dt.float8e4
DR = mybir.MatmulPerfMode.DoubleRow
ADD = mybir.AluOpType.add
SUB = mybir.AluOpType.subtract
IDENT = mybir.ActivationFunctionType.Identity
EXP = mybir.ActivationFunctionType.Exp
SQRT = mybir.ActivationFunctionType.Sqrt

_CACHE = {}


def build_bass():
    nc = bacc.Bacc(None, target_bir_lowering=False)

    x_h = nc.dram_tensor("xbf", [C, N], BF16, kind="ExternalInput")[:]
    # xdr[u, p, i, n] = x[u*256 + i*128 + p, n] in fp8 (DoubleRow layout)
    xdr_h = nc.dram_tensor("xdr", [U, P, 2, N], F8, kind="ExternalInput")[:]
    # wdr[wi, u, p, i, o] = w_wi[o, u*256 + i*128 + p]; wi order q,k,v,o
    w_h = nc.dram_tensor("wdr", [4, U, P, 2, C], BF16, kind="ExternalInput")[:]
    bq_h = nc.dram_tensor("bq", [C], F32, kind="ExternalInput")[:]
    bk_h = nc.dram_tensor("bk", [C], F32, kind="ExternalInput")[:]
    bv_h = nc.dram_tensor("bv", [C], F32, kind="ExternalInput")[:]
    bo_h = nc.dram_tensor("bo", [C], F32, kind="ExternalInput")[:]
    gam_h = nc.dram_tensor("gam", [C], F32, kind="ExternalInput")[:]
    bet_h = nc.dram_tensor("bet", [C], F32, kind="ExternalInput")[:]
    out_h = nc.dram_tensor("out", [C, NQ], F32, kind="ExternalOutput")[:]

    g8_np = np.zeros((P, NG), np.float32)
    g8T_np = np.zeros((NG, P), np.float32)
    for c in range(P):
        g8_np[c, c // GS] = 1.0 / GS
        g8T_np[c // GS, c] = 1.0
    g8_h = nc.inline_tensor(g8_np, name="g8")[:]
    g8T_h = nc.inline_tensor(g8T_np, name="g8T")[:]

    x_t = x_h.rearrange("(t p) n -> t p n", p=P)
    out_t = out_h.rearrange("(t p) n -> t p n", p=P)

    def col4(ap1d):
        return bass.AP(tensor=ap1d.tensor, offset=ap1d.offset, ap=[[1, P], [P, CT]])

    def col8(ap1d):
        return bass.AP(tensor=ap1d.tensor, offset=ap1d.offset, ap=[[1, 64], [64, 8]])

    with tile.TileContext(nc) as tc:
        with tc.tile_pool(name="consts", bufs=1) as cp, \
             tc.tile_pool(name="xbf", bufs=1) as xbp, \
             tc.tile_pool(name="xf8", bufs=1) as x8p, \
             tc.tile_pool(name="wf8", bufs=1) as w8p, \
             tc.tile_pool(name="qkv", bufs=1) as qkvp, \
             tc.tile_pool(name="hT", bufs=1) as hTp, \
             tc.tile_pool(name="pT", bufs=1) as pTp:

            # ---- constants ----
            ones8 = cp.tile([P, 2, 64], F8, tag="ones8")
            nc.vector.memset(ones8[:], 1.0)
            eps_t = cp.tile([P, 1], F32, tag="eps")
            nc.vector.memset(eps_t[:], EPS)
            mln16 = cp.tile([P, 1], F32, tag="mln16")
            nc.vector.memset(mln16[:], -LN16)
            g8_sb = cp.tile([P, NG], F32, tag="g8")
            nc.sync.dma_start(out=g8_sb[:], in_=g8_h)
            g8T_sb = cp.tile([NG, P], F32, tag="g8T")
            nc.sync.dma_start(out=g8T_sb[:], in_=g8T_h)
            gam_sb = cp.tile([P, CT], F32, tag="gam")
            bet_sb = cp.tile([P, CT], F32, tag="bet")
            bo4 = cp.tile([P, CT], F32, tag="bo4")
            bo4p = cp.tile([P, CT], F32, tag="bo4p")
            bv4 = cp.tile([P, CT], F32, tag="bv4")
            bv8c = cp.tile([P, CT], F8, tag="bv8c")
            bq8_0 = cp.tile([64, 8], F32, tag="bq8_0")
            bk8_0 = cp.tile([64, 8], F32, tag="bk8_0")
            bq8 = cp.tile([64, 8], F32, tag="bq8")
            bk8 = cp.tile([64, 8], F32, tag="bk8")
            bkc0 = cp.tile([P, CT], F32, tag="bkc0")
            bk4p = cp.tile([P, CT], F32, tag="bk4p")
            A4 = cp.tile([P, CT], F32, tag="A4")
            D4 = cp.tile([P, CT], F32, tag="D4")
            RA = cp.tile([P, CT], F32, tag="RA")
            DRA = cp.tile([P, CT], F32, tag="DRA")
            dfa8 = cp.tile([P, 2, U], F8, tag="dfa8")

            # ---- persistent activations ----
            x_bf = [xbp.tile([P, N], BF16, tag=f"x{t}", name=f"x{t}")
                    for t in range(CT)]
            xf8 = [x8p.tile([P, 2, N], F8, tag=f"x8{u}", name=f"x8{u}")
                   for u in range(U)]
            wf8 = [[w8p.tile([P, 2, C], F8, tag=f"w8_{w}{u}", name=f"w8_{w}{u}")
                    for u in range(U)] for w in range(3)]
            wo8p = [w8p.tile([P, C], F8, tag=f"wo8_{t}", name=f"wo8_{t}")
                    for t in range(CT)]
            q2 = [qkvp.tile([P, 2, NQ], F8, tag=f"q{u}", name=f"q{u}")
                  for u in range(U)]
            k2 = [qkvp.tile([P, 2, N], F8, tag=f"k{u}", name=f"k{u}")
                  for u in range(U)]
            v2 = [qkvp.tile([P, 2, C], F8, tag=f"v{m}", name=f"v{m}")
                  for m in range(M16)]
            hT2 = [hTp.tile([P, 2, NQ], F8, tag=f"hT{u}", name=f"hT{u}")
                   for u in range(U)]
            pT2 = [[pTp.tile([P, 2, 512], F8, tag=f"pT{b}_{m}",
                             name=f"pT{b}_{m}") for m in range(M16)]
                   for b in range(3)]

            # =========== Phase A/B ===========
            with tc.tile_pool(name="wbf", bufs=1) as wbp, \
                 tc.tile_pool(name="gn", bufs=2) as gnp, \
                 tc.tile_pool(name="gnps", bufs=2, space="PSUM") as gnps, \
                 tc.tile_pool(name="pj", bufs=6, space="PSUM") as pjp:

                w_bf = [[wbp.tile([P, 2, C], BF16, tag=f"wb{w}{u}",
                                  name=f"wb{w}{u}")
                         for u in range(U)] for w in range(4)]

                # x (fp8, host-formatted) first, then wk + gn affine vectors
                for ct in range(CT):
                    u, i = ct // 2, ct % 2
                    nc.sync.dma_start(out=xf8[u][:, i, :], in_=xdr_h[u][:, i, :])
                for u in range(U):
                    nc.sync.dma_start(out=w_bf[1][u][:], in_=w_h[1][u])
                nc.sync.dma_start(out=gam_sb[:], in_=col4(gam_h))
                nc.sync.dma_start(out=bet_sb[:], in_=col4(bet_h))

                stats = gnp.tile([P, CT, 8, 6], F32, tag="stats")
                asums = gnp.tile([P, 2], F32, tag="asums")
                ascr = gnp.tile([P, N], F8, tag="ascr")

                def gn_math(ct):
                    cstat = gnp.tile([P, 2], F32, tag="cstat", name=f"cs{ct}")
                    if ct == CT - 1:
                        # ct3 stats arrive as [sum(x), sum(x^2)] from ACT
                        nc.vector.tensor_scalar_mul(cstat[:], asums[:],
                                                    1.0 / N)
                    else:
                        mv = gnp.tile([P, 2], F32, tag="mv", name=f"mv{ct}")
                        nc.vector.bn_aggr(out=mv[:], in_=stats[:, ct, :, :])
                        nc.vector.tensor_copy(cstat[:, 0:1], mv[:, 0:1])
                        nc.vector.tensor_mul(cstat[:, 1:2], mv[:, 0:1],
                                             mv[:, 0:1])
                        nc.vector.tensor_add(cstat[:, 1:2], cstat[:, 1:2],
                                             mv[:, 1:2])
                    psA = gnps.tile([NG, 2], F32, tag="gn", name=f"gA{ct}")
                    nc.tensor.matmul(psA[:], lhsT=g8_sb[:], rhs=cstat[:],
                                     start=True, stop=True)
                    gt = gnp.tile([NG, 2], F32, tag="gt", name=f"gt{ct}")
                    nc.vector.tensor_copy(gt[:], psA[:])
                    psB = gnps.tile([P, 2], F32, tag="gn", name=f"gB{ct}")
                    nc.tensor.matmul(psB[:], lhsT=g8T_sb[:], rhs=gt[:],
                                     start=True, stop=True)
                    gstat = gnp.tile([P, 2], F32, tag="gstat", name=f"gs{ct}")
                    nc.vector.tensor_copy(gstat[:], psB[:])
                    # var + eps, then rsqrt via reciprocal seed + 2 Newton
                    # steps (avoids the Sqrt activation: keeping every ACT
                    # func in the exp table set avoids a mid-kernel
                    # LoadActFuncSet switch)
                    vtmp = gnp.tile([P, 1], F32, tag="vtmp", name=f"vt{ct}")
                    nc.vector.tensor_mul(vtmp[:], gstat[:, 0:1], gstat[:, 0:1])
                    nc.vector.tensor_tensor(out=vtmp[:], in0=gstat[:, 1:2],
                                            in1=vtmp[:], op=SUB)
                    nc.vector.tensor_scalar_add(out=vtmp[:], in0=vtmp[:],
                                                scalar1=EPS)
                    rstd = gnp.tile([P, 1], F32, tag="rstd", name=f"rs{ct}")
                    nc.vector.reciprocal(out=rstd[:], in_=vtmp[:])
                    nt = gnp.tile([P, 1], F32, tag="nt", name=f"nt{ct}")
                    for _ in range(2):
                        nc.vector.tensor_mul(nt[:], rstd[:], rstd[:])
                        nc.vector.tensor_mul(nt[:], nt[:], vtmp[:])
                        nc.vector.tensor_scalar(out=nt[:], in0=nt[:],
                                                scalar1=-0.5, scalar2=1.5,
                                                op0=mybir.AluOpType.mult,
                                                op1=ADD)
                        nc.vector.tensor_mul(rstd[:], rstd[:], nt[:])
                    nc.vector.tensor_mul(A4[:, ct:ct + 1], rstd[:],
                                         gam_sb[:, ct:ct + 1])
                    dt_ = gnp.tile([P, 1], F32, tag="dt", name=f"dt{ct}")
                    nc.vector.tensor_mul(dt_[:], gstat[:, 0:1], A4[:, ct:ct + 1])
                    nc.vector.tensor_tensor(out=D4[:, ct:ct + 1],
                                            in0=bet_sb[:, ct:ct + 1],
                                            in1=dt_[:], op=SUB)

                # stats straight off the fp8 x; gn math + wk-scale per chunk.
                # ct3's sums run on the otherwise-idle ACT engine so the
                # DVE-serial stats chain is ~25% shorter.
                nc.scalar.activation(out=ascr[:], in_=xf8[1][:, 1, :],
                                     func=mybir.ActivationFunctionType.Copy,
                                     accum_out=asums[:, 0:1])
                nc.scalar.activation(out=ascr[:], in_=xf8[1][:, 1, :],
                                     func=mybir.ActivationFunctionType.Square,
                                     accum_out=asums[:, 1:2])
                for ct in range(CT):
                    u, i = ct // 2, ct % 2
                    if ct < CT - 1:
                        for s8 in range(8):
                            nc.vector.bn_stats(
                                out=stats[:, ct, s8, :],
                                in_=xf8[u][:, i, s8 * 512:(s8 + 1) * 512],
                            )
                    gn_math(ct)
                    nc.vector.tensor_scalar_mul(wf8[1][u][:, i, :],
                                                w_bf[1][u][:, i, :],
                                                A4[:, ct:ct + 1])

                # remaining weights, residual x (bf16), small vectors
                for w in (2, 0, 3):
                    for u in range(U):
                        nc.sync.dma_start(out=w_bf[w][u][:], in_=w_h[w][u])
                nc.sync.dma_start(out=bo4[:], in_=col4(bo_h))
                nc.sync.dma_start(out=bv4[:], in_=col4(bv_h))
                nc.sync.dma_start(out=bq8_0[:], in_=col8(bq_h))
                nc.sync.dma_start(out=bk8_0[:], in_=col8(bk_h))
                nc.sync.dma_start(out=bkc0[:], in_=col4(bk_h))
                for ct in range(CT):
                    nc.sync.dma_start(out=x_bf[ct][:], in_=x_t[ct])

                nc.vector.reciprocal(out=RA[:], in_=A4[:])
                nc.gpsimd.tensor_mul(DRA[:], D4[:], RA[:])
                dsrc = DRA[:]
                nc.gpsimd.tensor_copy(
                    dfa8[:],
                    bass.AP(tensor=dsrc.tensor, offset=dsrc.offset,
                            ap=[dsrc.ap[0], [1, 2], [2, U]]),
                )
                nc.gpsimd.tensor_copy(bv8c[:], bv4[:])

                # wq / wv scaling and wo conversion on Pool
                for cc in range(CT):
                    u, i = cc // 2, cc % 2
                    acol = A4[:, cc:cc + 1]
                    nc.gpsimd.tensor_scalar_mul(wf8[0][u][:, i, :],
                                                w_bf[0][u][:, i, :], acol)
                    nc.gpsimd.tensor_scalar_mul(wf8[2][u][:, i, :],
                                                w_bf[2][u][:, i, :], acol)
                for cc in range(CT):
                    nc.gpsimd.tensor_copy(wo8p[cc][:],
                                          w_bf[3][cc // 2][:, cc % 2, :])

                # bias folds b' = b + w @ d in [64,8] block-column layout
                def bias_fold8(w, b0, bout, name):
                    for ob in range(8):
                        pf = gnps.tile([64, 1], F32, tag="gn",
                                       name=f"bf{name}{ob}")
                        for u in range(U):
                            nc.tensor.matmul(
                                pf[:],
                                lhsT=wf8[w][u][:, :, ob * 64:(ob + 1) * 64],
                                rhs=dfa8[:, :, u:u + 1],
                                start=(u == 0), stop=(u == U - 1),
                                perf_mode=DR)
                        nc.scalar.activation(out=bout[:, ob:ob + 1],
                                             in_=pf[:], func=IDENT,
                                             scale=1.0,
                                             bias=b0[:, ob:ob + 1])

                bias_fold8(1, bk8_0, bk8, "k")

                # plain-k blocks need the folded bias in [128,4] col layout
                for b in range(CT):
                    pf4 = gnps.tile([P, 1], F32, tag="gn", name=f"bk4_{b}")
                    for cc in range(CT):
                        nc.tensor.matmul(
                            pf4[:],
                            lhsT=wf8[1][cc // 2][:, cc % 2, b * P:(b + 1) * P],
                            rhs=dfa8[:, cc % 2, cc // 2:cc // 2 + 1],
                            start=(cc == 0), stop=(cc == CT - 1))
                    nc.scalar.activation(out=bk4p[:, b:b + 1], in_=pf4[:],
                                         func=IDENT, scale=1.0,
                                         bias=bkc0[:, b:b + 1])

                bias_fold8(0, bq8_0, bq8, "q")

                # bo' = bo + wo @ bv (plain fp8)
                for ct_o in range(CT):
                    pbo = gnps.tile([P, 1], F32, tag="gn", name=f"bo{ct_o}")
                    for cc in range(CT):
                        nc.tensor.matmul(
                            pbo[:],
                            lhsT=wo8p[cc][:, ct_o * P:(ct_o + 1) * P],
                            rhs=bv8c[:, cc:cc + 1],
                            start=(cc == 0), stop=(cc == CT - 1))
                    nc.scalar.activation(out=bo4p[:, ct_o:ct_o + 1],
                                         in_=pbo[:], func=IDENT, scale=1.0,
                                         bias=bo4[:, ct_o:ct_o + 1])

                # ---- k / v projections, interleaved (k evac ACT, v DVE) ----
                def emit_k_jsl(jsl):
                    tsl = slice(jsl * 512, (jsl + 1) * 512)
                    if jsl >= 6:        # plain fp8: full-width psum
                        for b in range(CT):
                            psk = pjp.tile([P, 512], F32, tag="pj",
                                           name=f"k{jsl}_{b}")
                            for cc in range(CT):
                                nc.tensor.matmul(
                                    psk[:],
                                    lhsT=wf8[1][cc // 2][:, cc % 2,
                                                         b * P:(b + 1) * P],
                                    rhs=xf8[cc // 2][:, cc % 2, tsl],
                                    start=(cc == 0), stop=(cc == CT - 1))
                            if b % 2 == 0:
                                nc.scalar.activation(
                                    out=k2[b // 2][:, b % 2, tsl], in_=psk[:],
                                    func=IDENT, scale=1.0,
                                    bias=bk4p[:, b:b + 1])
                            else:
                                nc.vector.tensor_scalar_add(
                                    out=k2[b // 2][:, b % 2, tsl],
                                    in0=psk[:], scalar1=bk4p[:, b:b + 1])
                    else:               # DoubleRow, evacs split ACT/DVE
                        for ob in range(8):
                            psk = pjp.tile([P, 512], F32, tag="pj",
                                           name=f"k{jsl}_{ob}")
                            for u in range(U):
                                nc.tensor.matmul(
                                    psk[0:64, :],
                                    lhsT=wf8[1][u][:, :, ob * 64:(ob + 1) * 64],
                                    rhs=xf8[u][:, :, tsl],
                                    start=(u == 0), stop=(u == U - 1),
                                    perf_mode=DR)
                            po = (ob % 2) * 64
                            dst = k2[ob // 4][po:po + 64, (ob % 4) // 2, tsl]
                            if ob % 2 == 0:
                                nc.scalar.activation(
                                    out=dst, in_=psk[0:64, :], func=IDENT,
                                    scale=1.0, bias=bk8[:, ob:ob + 1])
                            else:
                                nc.vector.tensor_scalar_add(
                                    out=dst, in0=psk[0:64, :],
                                    scalar1=bk8[:, ob:ob + 1])

                def emit_v_tb2(tb2, pool, tag):
                    m, i = tb2 // 2, tb2 % 2
                    if tb2 >= 18:       # plain fp8: full-width psum
                        psv = pool.tile([P, 512], F32, tag=tag,
                                        name=f"vp{tb2}")
                        t0 = tb2 * P
                        for cc in range(CT):
                            nc.tensor.matmul(
                                psv[:],
                                lhsT=xf8[cc // 2][:, cc % 2, t0:t0 + P],
                                rhs=wf8[2][cc // 2][:, cc % 2, :],
                                start=(cc == 0), stop=(cc == CT - 1))
                        nc.vector.tensor_copy(v2[m][:, i, :], psv[:])
                    else:               # DoubleRow, two 64-token halves
                        for hh in range(2):
                            psv = pool.tile([P, 512], F32, tag=tag,
                                            name=f"v{tb2}_{hh}")
                            t0 = tb2 * P + hh * 64
                            for u in range(U):
                                nc.tensor.matmul(
                                    psv[0:64, :],
                                    lhsT=xf8[u][:, :, t0:t0 + 64],
                                    rhs=wf8[2][u][:],
                                    start=(u == 0), stop=(u == U - 1),
                                    perf_mode=DR)
                            po = hh * 64
                            nc.vector.tensor_copy(v2[m][po:po + 64, i, :],
                                                  psv[0:64, :])

                for g in range(8):
                    emit_k_jsl(g)

                def emit_qproj(st, pool, tag, obs, on_act=False):
                    tsl = slice(st * 512, (st + 1) * 512)
                    for ob in obs:
                        psq = pool.tile([P, 512], F32, tag=tag,
                                        name=f"q{st}_{ob}")
                        for u in range(U):
                            nc.tensor.matmul(
                                psq[0:64, :],
                                lhsT=wf8[0][u][:, :, ob * 64:(ob + 1) * 64],
                                rhs=xf8[u][:, :, tsl],
                                start=(u == 0), stop=(u == U - 1),
                                perf_mode=DR)
                        po = (ob % 2) * 64
                        dst = q2[ob // 4][po:po + 64, (ob % 4) // 2, tsl]
                        if on_act:
                            nc.scalar.activation(out=dst, in_=psq[0:64, :],
                                                 func=IDENT, scale=1.0,
                                                 bias=bq8[:, ob:ob + 1])
                        else:
                            nc.vector.tensor_scalar_add(
                                out=dst, in0=psq[0:64, :],
                                scalar1=bq8[:, ob:ob + 1])

                emit_qproj(0, pjp, "pj", range(8), on_act=True)

            # =========== Phase C: attention strips ===========
            with tc.tile_pool(name="sc", bufs=7, space="PSUM") as scp, \
                 tc.tile_pool(name="lps", bufs=1, space="PSUM") as lp, \
                 tc.tile_pool(name="lsb", bufs=2) as lsp, \
                 tc.tile_pool(name="outt", bufs=3) as otp:

                rlb_of = {}
                psl_of = {}

                def emit_scores_unit(st, m, pT):
                    isl = slice(st * 512, (st + 1) * 512)
                    for jj in range(2):
                        jt = 2 * m + jj
                        if jt % 6 == 5:     # plain fp8 scores
                            ps = scp.tile([P, 512], F32, tag="sc",
                                          name=f"s{st}_{jt}")
                            j0 = jt * P
                            for cc in range(CT):
                                nc.tensor.matmul(
                                    ps[:],
                                    lhsT=k2[cc // 2][:, cc % 2, j0:j0 + P],
                                    rhs=q2[cc // 2][:, cc % 2, isl],
                                    start=(cc == 0), stop=(cc == CT - 1))
                            nc.scalar.activation(
                                out=pT[m][:, jj, :], in_=ps[:],
                                func=EXP, scale=SCALE, bias=mln16[:])
                        else:               # DoubleRow scores
                            for hh in range(2):
                                ps = scp.tile([P, 512], F32, tag="sc",
                                              name=f"s{st}_{jt}_{hh}")
                                j0 = jt * P + hh * 64
                                for u in range(U):
                                    nc.tensor.matmul(
                                        ps[0:64, :],
                                        lhsT=k2[u][:, :, j0:j0 + 64],
                                        rhs=q2[u][:, :, isl],
                                        start=(u == 0), stop=(u == U - 1),
                                        perf_mode=DR)
                                nc.scalar.activation(
                                    out=pT[m][hh * 64:(hh + 1) * 64, jj, :],
                                    in_=ps[0:64, :], func=EXP,
                                    scale=SCALE, bias=mln16[0:64, :])

                def emit_l_unit(st, m, pT):
                    # all-ones M=64 stationary: every psum row accumulates l,
                    # so the result is already broadcast across 64 partitions
                    nc.tensor.matmul(psl_of[st][0:64, :], lhsT=ones8[:],
                                     rhs=pT[m][:],
                                     start=(m == 0), stop=(m == M16 - 1),
                                     perf_mode=DR)

                def emit_rl_chain(st):
                    rlb = lsp.tile([64, 512], F32, tag="rlb", name=f"rlb{st}")
                    nc.vector.reciprocal(out=rlb[:], in_=psl_of[st][0:64, :])
                    rlb_of[st] = rlb

                def emit_h_chunk(sp, cb):
                    # h accumulation for channels cb*64.. of strip sp
                    i0 = sp * 512
                    pT = pT2[sp % 3]
                    psb_ = scp.tile([P, 512], F32, tag="sc",
                                    name=f"hB{sp}_{cb}")
                    for m in range(M16):
                        nc.tensor.matmul(
                            psb_[0:64, :],
                            lhsT=v2[m][:, :, cb * 64:(cb + 1) * 64],
                            rhs=pT[m][:],
                            start=(m == 0), stop=(m == M16 - 1),
                            perf_mode=DR)
                    po = (cb % 2) * 64
                    nc.vector.tensor_mul(
                        hT2[cb // 4][po:po + 64, (cb % 4) // 2, i0:i0 + 512],
                        psb_[0:64, :], rlb_of[sp][:])

                def emit_outproj(sp, blocks):
                    i0 = sp * 512
                    isl = slice(i0, i0 + 512)
                    for b in blocks:
                        pso = scp.tile([P, 512], F32, tag="sc",
                                       name=f"op{sp}_{b}")
                        for cc in range(CT):
                            nc.tensor.matmul(
                                pso[:],
                                lhsT=wo8p[cc][:, b * P:(b + 1) * P],
                                rhs=hT2[cc // 2][:, cc % 2, isl],
                                start=(cc == 0), stop=(cc == CT - 1))
                        ot = otp.tile([P, 512], F32, tag="ot",
                                      name=f"ot{sp}_{b}")
                        nc.vector.scalar_tensor_tensor(
                            out=ot[:], in0=pso[:], scalar=bo4p[:, b:b + 1],
                            in1=x_bf[b][:, isl], op0=ADD, op1=ADD)
                        nc.sync.dma_start(out=out_t[b][:, isl], in_=ot[:])

                hps3 = []

                for st in range(NSTRIP):
                    pT = pT2[st % 3]
                    psl_of[st] = lp.tile([P, 512], F32, tag="l",
                                         name=f"l{st}")
                    for m in range(M16):
                        if not (st > 0 and m < 2):
                            emit_scores_unit(st, m, pT)
                        if m >= 2:
                            emit_l_unit(st, m - 2, pT)
                        if st > 0:
                            if m == 1:
                                emit_rl_chain(st - 1)
                            elif 2 <= m <= 9:
                                emit_h_chunk(st - 1, m - 2)
                            elif m in (11, 12):
                                emit_outproj(st - 1, (2 * (m - 11),
                                                      2 * (m - 11) + 1))
                        if st == 0:
                            emit_v_tb2(2 * m, scp, "sc")
                            emit_v_tb2(2 * m + 1, scp, "sc")
                        if st < NSTRIP - 1 and m in (13, 14):
                            emit_qproj(st + 1, scp, "sc",
                                       range(4 * (m - 13), 4 * (m - 12)))
                        if st == NSTRIP - 1 and m >= 10:
                            # start the first 4 h chunks of the last strip
                            # in-strip: 2 m-steps per unit per chunk
                            if m == 10:
                                hps3.extend(
                                    scp.tile([P, 512], F32, tag="sc",
                                             name=f"h3_{cb}")
                                    for cb in range(4))
                            for cb in range(4):
                                for mm_ in (2 * (m - 10), 2 * (m - 10) + 1):
                                    nc.tensor.matmul(
                                        hps3[cb][0:64, :],
                                        lhsT=v2[mm_][:, :,
                                                     cb * 64:(cb + 1) * 64],
                                        rhs=pT[mm_][:],
                                        start=(mm_ == 0), stop=False,
                                        perf_mode=DR)
                    emit_l_unit(st, M16 - 2, pT)
                    emit_l_unit(st, M16 - 1, pT)
                    if st < NSTRIP - 1:
                        for m2 in range(2):
                            emit_scores_unit(st + 1, m2,
                                             pT2[(st + 1) % 3])

                # tail: last strip's h chunks with the out-proj matmuls
                # interleaved (each cc half of hT2 becomes ready after two
                # chunks), then fused evacuation + store
                sp = NSTRIP - 1
                i0 = sp * 512
                isl = slice(i0, i0 + 512)
                emit_rl_chain(sp)
                # finish chunks 0-3 (m-steps 12..15), then evacuate
                for cb in range(4):
                    for mm_ in range(12, M16):
                        nc.tensor.matmul(
                            hps3[cb][0:64, :],
                            lhsT=v2[mm_][:, :, cb * 64:(cb + 1) * 64],
                            rhs=pT2[sp % 3][mm_][:],
                            start=False, stop=(mm_ == M16 - 1),
                            perf_mode=DR)
                for cb in range(4):
                    po = (cb % 2) * 64
                    nc.vector.tensor_mul(
                        hT2[0][po:po + 64, (cb % 4) // 2, isl],
                        hps3[cb][0:64, :], rlb_of[sp][:])
                op_ps = [scp.tile([P, 512], F32, tag="sc", name=f"opt_{b}")
                         for b in range(CT)]
                # out-proj first half (contracts hT2[0], ready now)
                for cc in (0, 1):
                    for b in range(CT):
                        nc.tensor.matmul(
                            op_ps[b][:],
                            lhsT=wo8p[cc][:, b * P:(b + 1) * P],
                            rhs=hT2[0][:, cc, isl],
                            start=(cc == 0), stop=False)
                for cb in range(4, 8):
                    emit_h_chunk(sp, cb)
                for cc in (2, 3):
                    for b in range(CT):
                        nc.tensor.matmul(
                            op_ps[b][:],
                            lhsT=wo8p[cc][:, b * P:(b + 1) * P],
                            rhs=hT2[1][:, cc - 2, isl],
                            start=False, stop=(cc == CT - 1))
                for b in range(CT):
                    ot = otp.tile([P, 512], F32, tag="ot", name=f"ott_{b}")
                    nc.vector.scalar_tensor_tensor(
                        out=ot[:], in0=op_ps[b][:], scalar=bo4p[:, b:b + 1],
                        in1=x_bf[b][:, isl], op0=ADD, op1=ADD)
                    nc.sync.dma_start(out=out_t[b][:, isl], in_=ot[:])

    nc.finalize()
    return nc


def kernel(**inputs):
    if "nc" not in _CACHE:
        _CACHE["nc"] = build_bass()
    nc = _CACHE["nc"]

    x = np.asarray(inputs["x"], dtype=np.float32)
    B = x.shape[0]
    xf = x.reshape(B, C, N)

    def to_dr(w):
        wT = np.asarray(w, dtype=np.float32).T        # [c, o]
        return wT.reshape(U, 2, P, C).transpose(0, 2, 1, 3)

    wdr = np.ascontiguousarray(
        np.stack([to_dr(inputs[k]) for k in ("wq", "wk", "wv", "wo")])
        .astype(ml_dtypes.bfloat16)
    )

    shared = {
        "wdr": wdr,
        "bq": np.ascontiguousarray(np.asarray(inputs["bq"], np.float32)),
        "bk": np.ascontiguousarray(np.asarray(inputs["bk"], np.float32)),
        "bv": np.ascontiguousarray(np.asarray(inputs["bv"], np.float32)),
        "bo": np.ascontiguousarray(np.asarray(inputs["bo"], np.float32)),
        "gam": np.ascontiguousarray(np.asarray(inputs["norm_g"], np.float32)),
        "bet": np.ascontiguousarray(np.asarray(inputs["norm_b"], np.float32)),
    }

    in_maps = []
    for core in range(2 * B):
        b, half = core // 2, core % 2
        xb = xf[b]
        if half:
            xb = np.concatenate([xb[:, NQ:], xb[:, :NQ]], axis=1)
        # fp8 copy in DoubleRow layout [u, p, i, n]
        xdr = np.ascontiguousarray(
            xb.reshape(U, 2, P, N).transpose(0, 2, 1, 3)
            .astype(ml_dtypes.float8_e4m3))
        in_maps.append(
            {"xbf": np.ascontiguousarray(xb.astype(ml_dtypes.bfloat16)),
             "xdr": xdr, **shared})

    import os
    trace = bool(os.environ.get("BASS_KERNEL_TRACE"))
    res = run_bass_kernel_spmd(
        nc, in_maps, core_ids=list(range(2 * B)), trace=trace,
        trace_cores=list(range(2 * B)) if trace else None,
    )
    _CACHE["last_results"] = res

    out = np.empty((B, C, N), np.float32)
    for core in range(2 * B):
        b, half = core // 2, core % 2
        out[b][:, half * NQ:(half + 1) * NQ] = res.results[core]["out"]
    return out.reshape(B, C, 64, 64)
